# revision 1
# baseline (speedup 1.0000x reference)
"""Kernel builder for the dual-stream linear-attention transformer (per-core).

Layout convention:
  - "layout 1" activation: [E, N] feature-major; SBUF tiles [128, KE, C]
    (feature f = 128*k + p -> partition p, k-th slice; tokens on free dim).
  - "layout 2" activation: [N, E] token-major; SBUF tiles [128(tokens), E].
  - Residual streams live in internal DRAM as [E, N] (layout 1), streamed
    through SBUF in token chunks of C.

fp32r discipline (cfg.f32r): every matmul operand tile is declared
float32r. Producers are either DMA byte-casts (bitcast both sides) or DVE
ops (which round correctly on write). ACT must never WRITE an f32r tile
(hardware produces garbage); ACT/DVE readers view f32r tiles via
.bitcast(F32), which is exact.
"""

from dataclasses import dataclass
from contextlib import ExitStack

import numpy as np

import concourse.bass as bass
import concourse.mybir as mybir
import concourse.tile as tile

F32 = mybir.dt.float32
F32R = mybir.dt.float32r
AF = mybir.ActivationFunctionType
ALU = mybir.AluOpType

LN_EPS = 1e-5
BN_EPS = 1e-5


@dataclass
class Cfg:
    N: int = 2048
    E: int = 512
    R: int = 256
    X: int = 1024
    H: int = 8
    L: int = 3
    OUT: int = 15
    C: int = 512          # token chunk (free dim of layout-1 tiles)
    f32r: bool = True

    @property
    def KE(self):
        return self.E // 128

    @property
    def KR(self):
        return self.R // 128

    @property
    def KX(self):
        return self.X // 128

    @property
    def NC(self):
        return self.N // self.C

    @property
    def NTT(self):
        return self.C // 128  # token tiles per chunk


def host_constants(cfg):
    """Constant tensors passed as extra inputs (identical on every core)."""
    E, H = cfg.E, cfg.H
    dh = E // H
    ident = np.eye(128, dtype=np.float32)
    ones = np.ones((128, 128), dtype=np.float32)
    hmask = np.zeros((E, H), dtype=np.float32)
    for f in range(E):
        hmask[f, f // dh] = 1.0
    cmask = hmask.T.copy()
    return {"ident": ident, "ones128": ones, "hmask": hmask, "cmask": cmask}


PHASES = []


def build(nc, cfg):
    """Declare IO and build the whole program inside a TileContext."""
    c = cfg
    E, R, X, H, N, C, L = c.E, c.R, c.X, c.H, c.N, c.C, c.L
    KE, KR, KX, NC, NTT = c.KE, c.KR, c.KX, c.NC, c.NTT
    E4, E2, E8 = E // 4, E // 2, E // 8

    dt = F32
    MMDT = F32R if c.f32r else F32
    din = {}

    def inp(name, shape):
        din[name] = nc.dram_tensor(name, list(shape), dt, kind="ExternalInput")
        return din[name].ap()

    body_feats = inp("body_feats", (N, E))
    limb_feats = inp("limb_feats", (N, E))
    dw = inp("dw", (L, 4, 3, E, R))
    uw = inp("uw", (L, 4, 3, R, E))
    ub = inp("ub", (L, 4, 3, E))
    ow = inp("ow", (L, 4, E, E))
    ob = inp("ob", (L, 4, E))
    w1 = inp("w1", (L, 2, E, X))
    b1 = inp("b1", (L, 2, X))
    cw = inp("cw", (L, 2, X, 3))
    cb = inp("cb", (L, 2, X))
    bng = inp("bng", (L, 2, X))
    bnb = inp("bnb", (L, 2, X))
    w2 = inp("w2", (L, 2, X, E))
    b2 = inp("b2", (L, 2, E))
    lng = inp("lng", (L, 5, E))
    lnb = inp("lnb", (L, 5, E))
    gw1 = inp("gw1", (L, 2 * E, E4))
    gb1 = inp("gb1", (L, E4))
    gw2 = inp("gw2", (L, E4, 2))
    gb2 = inp("gb2", (L, 2))
    fw1 = inp("fw1", (2 * E, E2))
    fb1 = inp("fb1", (E2,))
    fw2 = inp("fw2", (E2, E))
    fb2 = inp("fb2", (E,))
    flng = inp("flng", (E,))
    flnb = inp("flnb", (E,))
    rw1 = inp("rw1", (E, E4))
    rb1 = inp("rb1", (E4,))
    rw2 = inp("rw2", (E4, E8))
    rb2 = inp("rb2", (E8,))
    rw3 = inp("rw3", (E8, c.OUT))
    rb3 = inp("rb3", (c.OUT,))
    ident_in = inp("ident", (128, 128))
    ones_in = inp("ones128", (128, 128))
    hmask_in = inp("hmask", (E, H))
    cmask_in = inp("cmask", (H, E))

    out_dram = nc.dram_tensor("out", [N, c.OUT], dt, kind="ExternalOutput")

    def idram(name):
        return nc.dram_tensor(name, [E, N], dt).ap().rearrange(
            "(k p) n -> p k n", p=128)

    rs = {}
    for s in ("b", "l"):
        rs[s, 0] = idram(f"r{s}0")
        for l in range(L):
            for st in (1, 2, 3):
                rs[s, (l, st)] = idram(f"r{s}_{l}_{st}")

    lowp = nc.allow_low_precision("f32r matmul operand rounding")

    with tile.TileContext(nc) as tc, ExitStack() as ctx, lowp:
        p_ = ctx.enter_context
        cst = p_(tc.tile_pool(name="cst", bufs=1))
        wbig = p_(tc.tile_pool(name="wbig", bufs=3))
        wsm = p_(tc.tile_pool(name="wsm", bufs=2))
        wcol = p_(tc.tile_pool(name="wcol", bufs=10))
        wrow = p_(tc.tile_pool(name="wrow", bufs=3))
        pa = p_(tc.tile_pool(name="pa", bufs=7))      # 8KB tiles
        pb = p_(tc.tile_pool(name="pb", bufs=3))      # 16KB tiles
        pc = p_(tc.tile_pool(name="pc", bufs=5))      # 2KB tiles
        pat = p_(tc.tile_pool(name="pat", bufs=2))    # per-attn persistents
        phl = p_(tc.tile_pool(name="phl", bufs=3))    # conv halos
        ps = p_(tc.tile_pool(name="ps", bufs=5, space="PSUM"))
        ps1 = p_(tc.tile_pool(name="ps1", bufs=2, space="PSUM"))
        psb = p_(tc.tile_pool(name="psb", bufs=1, space="PSUM"))

        v, sc, gp = nc.vector, nc.scalar, nc.gpsimd

        def mm(out, lhsT, rhs, start, stop):
            nc.tensor.matmul(out, lhsT, rhs, start=start, stop=stop)

        def F(ap):
            """fp32 view of an f32r tile (exact)."""
            return ap.bitcast(F32) if c.f32r else ap

        def M(ap):
            """f32r byte-view of an fp32 DRAM AP (for DMA byte-casts)."""
            return ap.bitcast(MMDT) if c.f32r else ap

        # ---- constants ----
        ident_t = cst.tile([128, 128], dt, tag="ident")
        nc.sync.dma_start(out=ident_t, in_=ident_in)
        ones_t = cst.tile([128, 128], MMDT, tag="ones")
        nc.sync.dma_start(out=ones_t, in_=M(ones_in))
        hmask_t = cst.tile([128, KE, H], dt, tag="hmask")
        nc.sync.dma_start(out=hmask_t,
                          in_=hmask_in.rearrange("(k p) h -> p k h", p=128))
        cmask_t = cst.tile([H, KE, 128], MMDT, tag="cmask")
        nc.sync.dma_start(out=cmask_t,
                          in_=M(cmask_in.rearrange("h (k p) -> h k p", p=128)))
        ONES_COL = ones_t[:, 0:1]
        ONES_ROW = ones_t[0:1, :]
        eps_den = cst.tile([8, 1], dt, tag="epsd")
        v.memset(eps_den, 1e-6)
        eps_ln = cst.tile([1, 1], dt, tag="epsl")
        v.memset(eps_ln, LN_EPS)

        def col_tile(src_ap, m, tag="col"):
            t = wcol.tile([128, m], dt, tag=tag)
            nc.sync.dma_start(out=t, in_=src_ap.rearrange("(m p) -> p m", p=128))
            return t

        def ln_stats_apply(xs, g_col, b_col, out_tiles, relu=False,
                           out_f32r=False):
            """LayerNorm over features (layout 1). xs: KE f32r APs [128, C]."""
            sq = pa.tile([128, KE, C], MMDT, tag="a8")
            for m in range(KE):
                v.tensor_tensor(out=sq[:, m, :], in0=F(xs[m]), in1=F(xs[m]),
                                op=ALU.mult)
            ps_s = ps1.tile([1, C], F32, tag="st")
            ps_ss = ps1.tile([1, C], F32, tag="st")
            for m in range(KE):
                mm(ps_s, ONES_COL, xs[m], start=(m == 0), stop=(m == KE - 1))
                mm(ps_ss, ONES_COL, sq[:, m, :], start=(m == 0),
                   stop=(m == KE - 1))
            arow = pc.tile([1, C], dt, tag="a2")   # mean
            brow = pc.tile([1, C], dt, tag="a2")   # msq -> var -> sd
            trow2 = pc.tile([1, C], dt, tag="a2")  # mean^2
            sc.activation(arow, ps_s, AF.Copy, scale=1.0 / E)
            sc.activation(brow, ps_ss, AF.Copy, scale=1.0 / E)
            sc.activation(trow2, arow, AF.Square)
            v.tensor_tensor(out=brow, in0=brow, in1=trow2, op=ALU.subtract)
            sc.activation(brow, brow, AF.Sqrt, bias=eps_ln[0:1, 0:1])
            srow = pc.tile([1, C], MMDT, tag="a2")
            v.reciprocal(out=srow, in_=brow)
            trow = pc.tile([1, C], MMDT, tag="a2")
            v.tensor_tensor(out=trow, in0=arow, in1=F(srow), op=ALU.mult)
            bc_s = psb.tile([128, C], F32, tag="bc")
            mm(bc_s, ONES_ROW, srow, start=True, stop=True)
            sb_s = pc.tile([128, C], dt, tag="a2")
            sc.activation(sb_s, bc_s, AF.Copy)
            bc_t = psb.tile([128, C], F32, tag="bc")
            mm(bc_t, ONES_ROW, trow, start=True, stop=True)
            sb_t = pc.tile([128, C], dt, tag="a2")
            sc.activation(sb_t, bc_t, AF.Copy)
            tmp = pa.tile([128, KE, C], dt, tag="a8")
            for m in range(KE):
                gp.tensor_tensor(out=tmp[:, m, :], in0=F(xs[m]), in1=sb_s,
                                 op=ALU.mult)
                gp.tensor_tensor(out=tmp[:, m, :], in0=tmp[:, m, :], in1=sb_t,
                                 op=ALU.subtract)
                if not out_f32r:
                    fn = AF.Relu if relu else AF.Identity
                    sc.activation(out_tiles[m], tmp[:, m, :], fn,
                                  bias=b_col[:, m:m + 1],
                                  scale=g_col[:, m:m + 1])
                elif relu:
                    tmpf = pc.tile([128, C], dt, tag="a2")
                    v.tensor_scalar(out=tmpf, in0=tmp[:, m, :],
                                    scalar1=g_col[:, m:m + 1],
                                    scalar2=b_col[:, m:m + 1],
                                    op0=ALU.mult, op1=ALU.add)
                    v.tensor_scalar_max(out_tiles[m], tmpf, 0.0)
                else:
                    v.tensor_scalar(out=out_tiles[m], in0=tmp[:, m, :],
                                    scalar1=g_col[:, m:m + 1],
                                    scalar2=b_col[:, m:m + 1],
                                    op0=ALU.mult, op1=ALU.add)

        def load_x_chunk(dram_l1, ci, tag="a8"):
            xt = pa.tile([128, KE, C], MMDT, tag=tag)
            nc.sync.dma_start(out=xt,
                              in_=M(dram_l1[:, :, ci * C:(ci + 1) * C]))
            return xt

        def store_chunk(dram_l1, ci, t):
            # stores ride the GPSIMD SWDGE queue so they never delay the
            # SP-queue loads that sit on the next phase's critical path
            gp.dma_start(out=dram_l1[:, :, ci * C:(ci + 1) * C], in_=t)

        # ---- entry transpose ----
        def entry(x_ap, dst):
            for ttk in range(N // 128):
                x2 = pa.tile([128, E], dt, tag="a8")
                nc.sync.dma_start(out=x2, in_=x_ap[ttk * 128:(ttk + 1) * 128, :])
                xt = pa.tile([128, KE, 128], dt, tag="a8")
                for f in range(KE):
                    pt = ps.tile([128, 128], F32, tag="mm")
                    nc.tensor.transpose(pt, x2[:, f * 128:(f + 1) * 128],
                                        ident_t)
                    sc.activation(xt[:, f, :], pt, AF.Copy)
                nc.sync.dma_start(out=dst[:, :, ttk * 128:(ttk + 1) * 128],
                                  in_=xt)

        PHASES.append(("entry", len(nc.inst_map)))
        entry(body_feats, rs["b", 0])
        entry(limb_feats, rs["l", 0])

        # ---- linear attention ----
        def attn(l, a, xq_dram, xkv_dram, tail):
            """tail(ci, proj_tiles(f32r, ob added), xq(f32r))."""
            dwt = wbig.tile([128, KE, 3, R], MMDT, tag="w")
            for t3 in range(3):
                nc.sync.dma_start(
                    out=dwt[:, :, t3, :],
                    in_=M(dw[l, a, t3].rearrange("(k p) r -> p k r", p=128)))
            uwt = wbig.tile([128, KR, 3, E], MMDT, tag="w")
            for t3 in range(3):
                nc.sync.dma_start(
                    out=uwt[:, :, t3, :],
                    in_=M(uw[l, a, t3].rearrange("(k p) e -> p k e", p=128)))
            owt = wbig.tile([128, KE, E], MMDT, tag="w")
            nc.sync.dma_start(
                out=owt, in_=M(ow[l, a].rearrange("(k p) e -> p k e", p=128)))
            ubq_col = col_tile(ub[l, a, 0], KE)
            ubk_row = wrow.tile([1, E], MMDT, tag="row")
            nc.sync.dma_start(out=ubk_row, in_=M(ub[l, a, 1][None, :]))
            ubv_row = wrow.tile([1, E], MMDT, tag="row")
            nc.sync.dma_start(out=ubv_row, in_=M(ub[l, a, 2][None, :]))
            ob_col = col_tile(ob[l, a], KE)

            PHASES.append((f"attn{l}.{a}.alpha", len(nc.inst_map)))
            kv_acc = pat.tile([128, 4, 258], dt, tag="kva")

            # alpha: k/v -> kv, ksum (ones column appended to v)
            for ci in range(NC):
                xt = load_x_chunk(xkv_dram, ci)
                lowk = pa.tile([128, KR, C], MMDT, tag="a8")
                lowv = pa.tile([128, KR, C], MMDT, tag="a8")
                for t, low in ((1, lowk), (2, lowv)):
                    pls = [ps.tile([128, C], F32, tag="mm", name=f"pl{_i}")
                           for _i in range(KR)]
                    for k in range(KE):
                        for m in range(KR):
                            mm(pls[m], dwt[:, k, t, m * 128:(m + 1) * 128],
                               xt[:, k, :], start=(k == 0), stop=(k == KE - 1))
                    for m in range(KR):
                        v.tensor_copy(low[:, m, :], pls[m])
                k2f = pa.tile([128, NTT, E], MMDT, tag="a8")
                v2x = pa.tile([128, NTT, 2, 258], MMDT, tag="a8")
                v.memset(F(v2x[:, :, :, 256:258]), 1.0)
                for tt in range(NTT):
                    pk = ps.tile([128, E], F32, tag="mm")
                    pv = ps.tile([128, E], F32, tag="mm")
                    for k in range(KR):
                        mm(pk, lowk[:, k, tt * 128:(tt + 1) * 128],
                           uwt[:, k, 1, :], start=(k == 0), stop=False)
                        mm(pv, lowv[:, k, tt * 128:(tt + 1) * 128],
                           uwt[:, k, 2, :], start=(k == 0), stop=False)
                    mm(pk, ONES_ROW, ubk_row, start=False, stop=True)
                    mm(pv, ONES_ROW, ubv_row, start=False, stop=True)
                    ee = pc.tile([128, E], dt, tag="a2")
                    rr = pc.tile([128, E], dt, tag="a2")
                    sc.activation(ee, pk, AF.Exp)
                    sc.activation(rr, pk, AF.Relu)
                    gp.tensor_scalar_min(ee, ee, 1.0)
                    v.tensor_tensor(out=k2f[:, tt, :], in0=ee, in1=rr,
                                    op=ALU.add)
                    v.tensor_copy(v2x[:, tt, 0, 0:256], pv[:, 0:256])
                    v.tensor_copy(v2x[:, tt, 1, 0:256], pv[:, 256:512])
                pkvs = [ps.tile([128, 258], F32, tag="mm", name=f"pkv{_i}")
                        for _i in range(4)]
                for tt in range(NTT):
                    for p in range(4):
                        mm(pkvs[p], k2f[:, tt, p * 128:(p + 1) * 128],
                           v2x[:, tt, p // 2, :],
                           start=(tt == 0), stop=(tt == NTT - 1))
                for p in range(4):
                    if ci == 0:
                        sc.activation(kv_acc[:, p, :], pkvs[p], AF.Copy)
                    else:
                        v.tensor_tensor(out=kv_acc[:, p, :],
                                        in0=kv_acc[:, p, :], in1=pkvs[p],
                                        op=ALU.add)

            bd = pat.tile([128, KE, 128], MMDT, tag="bd")
            v.memset(F(bd), 0.0)
            for p in range(4):
                h0c = (2 * p % 4) * 64
                h1c = ((2 * p + 1) % 4) * 64
                v.tensor_copy(bd[0:64, p, 0:64], kv_acc[0:64, p, h0c:h0c + 64])
                v.tensor_copy(bd[64:128, p, 64:128],
                              kv_acc[64:128, p, h1c:h1c + 64])
            kmm = pat.tile([128, KE, H], MMDT, tag="km")
            for k in range(KE):
                v.tensor_scalar_mul(kmm[:, k, :], hmask_t[:, k, :],
                                    kv_acc[:, k, 256:257])

            # beta: q -> attention out-proj
            PHASES.append((f"attn{l}.{a}.beta", len(nc.inst_map)))
            for ci in range(NC):
                xq = load_x_chunk(xq_dram, ci)
                lowq = pa.tile([128, KR, C], MMDT, tag="a8")
                pls = [ps.tile([128, C], F32, tag="mm", name=f"plq{_i}") for _i in range(KR)]
                for k in range(KE):
                    for m in range(KR):
                        mm(pls[m], dwt[:, k, 0, m * 128:(m + 1) * 128],
                           xq[:, k, :], start=(k == 0), stop=(k == KE - 1))
                for m in range(KR):
                    v.tensor_copy(lowq[:, m, :], pls[m])
                qf = pa.tile([128, KE, C], MMDT, tag="a8")
                pqs = [ps.tile([128, C], F32, tag="mm", name=f"pq{_i}") for _i in range(KE)]
                for k in range(KR):
                    for m in range(KE):
                        mm(pqs[m], uwt[:, k, 0, m * 128:(m + 1) * 128],
                           lowq[:, k, :], start=(k == 0), stop=(k == KR - 1))
                for m in range(KE):
                    ee = pc.tile([128, C], dt, tag="a2")
                    rr = pc.tile([128, C], dt, tag="a2")
                    sc.activation(ee, pqs[m], AF.Exp, bias=ubq_col[:, m:m + 1])
                    sc.activation(rr, pqs[m], AF.Relu, bias=ubq_col[:, m:m + 1])
                    gp.tensor_scalar_min(ee, ee, 1.0)
                    v.tensor_tensor(out=qf[:, m, :], in0=ee, in1=rr, op=ALU.add)
                pd = ps.tile([8, C], F32, tag="mm")
                for k in range(KE):
                    mm(pd, kmm[:, k, :], qf[:, k, :], start=(k == 0),
                       stop=(k == KE - 1))
                den = pc.tile([8, C], dt, tag="a2")
                sc.activation(den, pd, AF.Identity, bias=eps_den)
                rec = pc.tile([8, C], MMDT, tag="a2")
                v.reciprocal(out=rec, in_=den)
                att = pa.tile([128, KE, C], MMDT, tag="a8")
                for m in range(KE):
                    pn = ps.tile([128, C], F32, tag="mm")
                    mm(pn, bd[:, m, :], qf[:, m, :], start=True, stop=True)
                    pr = ps.tile([128, C], F32, tag="mm")
                    mm(pr, cmask_t[:, m, :], rec, start=True, stop=True)
                    rb = pc.tile([128, C], dt, tag="a2")
                    sc.activation(rb, pr, AF.Copy)
                    v.tensor_tensor(out=att[:, m, :], in0=pn, in1=rb,
                                    op=ALU.mult)
                proj = pa.tile([128, KE, C], MMDT, tag="a8")
                pos = [ps.tile([128, C], F32, tag="mm", name=f"po{_i}") for _i in range(KE)]
                for k in range(KE):
                    for m in range(KE):
                        mm(pos[m], owt[:, k, m * 128:(m + 1) * 128],
                           att[:, k, :], start=(k == 0), stop=(k == KE - 1))
                for m in range(KE):
                    v.tensor_scalar_add(proj[:, m, :], pos[m],
                                        ob_col[:, m:m + 1])
                tail(ci, proj, xq)

        # ---- tails ----
        def make_self_tail(l, s, dst):
            g_col = col_tile(lng[l, 0 if s == "b" else 1], KE, tag="lncol")
            b_col = col_tile(lnb[l, 0 if s == "b" else 1], KE, tag="lncol")

            def tail(ci, proj, xq):
                for m in range(KE):
                    v.tensor_tensor(out=proj[:, m, :], in0=F(proj[:, m, :]),
                                    in1=F(xq[:, m, :]), op=ALU.add)
                outt = pa.tile([128, KE, C], dt, tag="a8")
                ln_stats_apply([proj[:, m, :] for m in range(KE)], g_col, b_col,
                               [outt[:, m, :] for m in range(KE)])
                store_chunk(dst, ci, outt)

            return tail

        def make_cross_tail(l, s, dst):
            gw1t = wsm.tile([128, 2 * KE, E4], MMDT, tag="ws")
            nc.sync.dma_start(out=gw1t,
                              in_=M(gw1[l].rearrange("(k p) g -> p k g", p=128)))
            gw2t = wsm.tile([128, 2], dt, tag="ws")
            nc.sync.dma_start(out=gw2t, in_=gw2[l])
            gwd = pat.tile([128, 1], MMDT, tag="gwd")
            v.tensor_tensor(out=gwd, in0=gw2t[:, 0:1], in1=gw2t[:, 1:2],
                            op=ALU.subtract)
            gb1_col = col_tile(gb1[l], 1, tag="lncol")
            gb2a = pat.tile([1, 1], dt, tag="gb2")
            nc.sync.dma_start(out=gb2a, in_=gb2[l, 0:1][None, :])
            gb2b = pat.tile([1, 1], dt, tag="gb2b")
            nc.sync.dma_start(out=gb2b, in_=gb2[l, 1:2][None, :])
            gb2d = pat.tile([1, 1], dt, tag="gb2d")
            v.tensor_tensor(out=gb2d, in0=gb2a, in1=gb2b, op=ALU.subtract)
            g_col = col_tile(lng[l, 2], KE, tag="lncol")
            b_col = col_tile(lnb[l, 2], KE, tag="lncol")

            def tail(ci, proj, xq):
                pg = ps.tile([128, C], F32, tag="mm")
                for k in range(2 * KE):
                    rhs = xq[:, k, :] if k < KE else proj[:, k - KE, :]
                    mm(pg, gw1t[:, k, :], rhs, start=(k == 0),
                       stop=(k == 2 * KE - 1))
                g1f = pc.tile([128, C], dt, tag="a2")
                v.tensor_scalar(out=g1f, in0=pg, scalar1=gb1_col[:, 0:1],
                                scalar2=0.0, op0=ALU.add, op1=ALU.max)
                g1t = pc.tile([128, C], MMDT, tag="a2")
                v.tensor_scalar_min(g1t, g1f, 6.0)
                pg2 = ps.tile([1, C], F32, tag="mm")
                mm(pg2, gwd, g1t, start=True, stop=True)
                bgf = pc.tile([1, C], dt, tag="a2")
                sc.activation(bgf, pg2, AF.Sigmoid, bias=gb2d[0:1, 0:1])
                bg = pc.tile([1, C], MMDT, tag="a2")
                v.tensor_copy(bg, bgf)
                pbg = psb.tile([128, C], F32, tag="bc")
                mm(pbg, ONES_ROW, bg, start=True, stop=True)
                mt = pa.tile([128, KE, C], MMDT, tag="a8")
                for m in range(KE):
                    dtmp = pc.tile([128, C], dt, tag="a2")
                    gp.tensor_tensor(out=dtmp, in0=F(xq[:, m, :]),
                                     in1=F(proj[:, m, :]), op=ALU.subtract)
                    v.tensor_tensor(out=dtmp, in0=dtmp, in1=pbg, op=ALU.mult)
                    v.tensor_tensor(out=mt[:, m, :], in0=dtmp,
                                    in1=F(proj[:, m, :]), op=ALU.add)
                outt = pa.tile([128, KE, C], dt, tag="a8")
                ln_stats_apply([mt[:, m, :] for m in range(KE)], g_col, b_col,
                               [outt[:, m, :] for m in range(KE)])
                store_chunk(dst, ci, outt)

            return tail

        # ---- FFN ----
        def ffn(l, s, src, dst):
            PHASES.append((f"ffn{l}.{s}", len(nc.inst_map)))
            si = 0 if s == "b" else 1
            w1t = wbig.tile([128, KE, X], MMDT, tag="w")
            nc.sync.dma_start(
                out=w1t, in_=M(w1[l, si].rearrange("(k p) x -> p k x", p=128)))
            w2t = wbig.tile([128, KX, E], MMDT, tag="w")
            nc.sync.dma_start(
                out=w2t, in_=M(w2[l, si].rearrange("(k p) e -> p k e", p=128)))
            b1_col = col_tile(b1[l, si], KX, tag="ffcol")
            b2_col = col_tile(b2[l, si], KE, tag="ffcol")
            w0_col = col_tile(cw[l, si, :, 0], KX, tag="ffcol")
            w1c_col = col_tile(cw[l, si, :, 1], KX, tag="ffcol")
            w2_col = col_tile(cw[l, si, :, 2], KX, tag="ffcol")
            cb_col = col_tile(cb[l, si], KX, tag="ffcol")
            bng_col = col_tile(bng[l, si], KX, tag="ffcol")
            bnb_col = col_tile(bnb[l, si], KX, tag="ffcol")
            rsq = float(1.0 / np.sqrt(1.0 + BN_EPS))
            A_col = wcol.tile([128, KX], dt, tag="ffcol")
            sc.activation(A_col, bng_col, AF.Copy, scale=rsq)
            B_col = wcol.tile([128, KX], dt, tag="ffcol")
            v.tensor_tensor(out=B_col, in0=cb_col, in1=A_col, op=ALU.mult)
            v.tensor_tensor(out=B_col, in0=B_col, in1=bnb_col, op=ALU.add)
            g_col = col_tile(lng[l, 3 if s == "b" else 4], KE, tag="lncol")
            bb_col = col_tile(lnb[l, 3 if s == "b" else 4], KE, tag="lncol")

            hts = [None] * NC
            xts = [None] * NC
            hl0 = [None] * NC   # last col scaled by w0
            hf2 = [None] * NC   # first col scaled by w2

            def compute_h(ci):
                xt = load_x_chunk(src, ci)
                xts[ci] = xt
                ht = pb.tile([128, KX, C], dt, tag="a16")
                for g in range(2):
                    phs = [ps.tile([128, C], F32, tag="mm", name=f"ph{_i}") for _i in range(4)]
                    for k in range(KE):
                        for j in range(4):
                            m = g * 4 + j
                            mm(phs[j], w1t[:, k, m * 128:(m + 1) * 128],
                               xt[:, k, :], start=(k == 0),
                               stop=(k == KE - 1))
                    for j in range(4):
                        m = g * 4 + j
                        sc.activation(ht[:, m, :], phs[j], AF.Relu,
                                      bias=b1_col[:, m:m + 1])
                        gp.tensor_scalar_min(ht[:, m, :], ht[:, m, :], 6.0)
                hts[ci] = ht
                l0 = phl.tile([128, KX, 1], dt, tag="hl")
                f2 = phl.tile([128, KX, 1], dt, tag="hf")
                for m in range(KX):
                    sc.activation(l0[:, m, :], ht[:, m, C - 1:C], AF.Copy,
                                  scale=w0_col[:, m:m + 1])
                    sc.activation(f2[:, m, :], ht[:, m, 0:1], AF.Copy,
                                  scale=w2_col[:, m:m + 1])
                hl0[ci], hf2[ci] = l0, f2

            def conv_tail(ci):
                ht = hts[ci]
                h2 = pb.tile([128, KX, C], MMDT, tag="a16")
                for m in range(KX):
                    acc = pc.tile([128, C], dt, tag="a2")
                    tmp = pc.tile([128, C], dt, tag="a2")
                    sc.activation(acc, ht[:, m, :], AF.Copy,
                                  scale=w1c_col[:, m:m + 1])
                    sc.activation(tmp, ht[:, m, :], AF.Copy,
                                  scale=w0_col[:, m:m + 1])
                    gp.tensor_tensor(out=acc[:, 1:C], in0=acc[:, 1:C],
                                     in1=tmp[:, 0:C - 1], op=ALU.add)
                    if ci > 0:
                        gp.tensor_tensor(out=acc[:, 0:1], in0=acc[:, 0:1],
                                         in1=hl0[ci - 1][:, m, :], op=ALU.add)
                    sc.activation(tmp, ht[:, m, :], AF.Copy,
                                  scale=w2_col[:, m:m + 1])
                    gp.tensor_tensor(out=acc[:, 0:C - 1], in0=acc[:, 0:C - 1],
                                     in1=tmp[:, 1:C], op=ALU.add)
                    if ci < NC - 1:
                        gp.tensor_tensor(out=acc[:, C - 1:C],
                                         in0=acc[:, C - 1:C],
                                         in1=hf2[ci + 1][:, m, :], op=ALU.add)
                    acc2 = pc.tile([128, C], dt, tag="a2")
                    sc.activation(acc2, acc, AF.Relu,
                                  scale=A_col[:, m:m + 1],
                                  bias=B_col[:, m:m + 1])
                    v.tensor_scalar_min(h2[:, m, :], acc2, 6.0)
                rt = pa.tile([128, KE, C], MMDT, tag="a8")
                pws = [ps.tile([128, C], F32, tag="mm", name=f"pw{_i}") for _i in range(KE)]
                for k in range(KX):
                    for m in range(KE):
                        mm(pws[m], w2t[:, k, m * 128:(m + 1) * 128],
                           h2[:, k, :], start=(k == 0), stop=(k == KX - 1))
                for m in range(KE):
                    rtf = pc.tile([128, C], dt, tag="a2")
                    sc.activation(rtf, pws[m], AF.Identity,
                                  bias=b2_col[:, m:m + 1])
                    v.tensor_tensor(out=rt[:, m, :], in0=rtf,
                                    in1=F(xts[ci][:, m, :]), op=ALU.add)
                outt = pa.tile([128, KE, C], dt, tag="a8")
                ln_stats_apply([rt[:, m, :] for m in range(KE)], g_col, bb_col,
                               [outt[:, m, :] for m in range(KE)])
                store_chunk(dst, ci, outt)
                hts[ci] = xts[ci] = None

            compute_h(0)
            for ci in range(1, NC):
                compute_h(ci)
                conv_tail(ci - 1)
            conv_tail(NC - 1)

        # ---- layers ----
        for l in range(L):
            bsrc = rs["b", 0] if l == 0 else rs["b", (l - 1, 3)]
            lsrc = rs["l", 0] if l == 0 else rs["l", (l - 1, 3)]
            attn(l, 0, bsrc, bsrc, make_self_tail(l, "b", rs["b", (l, 1)]))
            attn(l, 1, lsrc, lsrc, make_self_tail(l, "l", rs["l", (l, 1)]))
            attn(l, 2, rs["b", (l, 1)], rs["l", (l, 1)],
                 make_cross_tail(l, "b", rs["b", (l, 2)]))
            attn(l, 3, rs["l", (l, 1)], rs["b", (l, 1)],
                 make_cross_tail(l, "l", rs["l", (l, 2)]))
            ffn(l, "b", rs["b", (l, 2)], rs["b", (l, 3)])
            ffn(l, "l", rs["l", (l, 2)], rs["l", (l, 3)])

        PHASES.append(("final", len(nc.inst_map)))
        # ---- final head ----
        fw1t = wbig.tile([128, 2 * KE, E2], MMDT, tag="w")
        nc.sync.dma_start(out=fw1t,
                          in_=M(fw1.rearrange("(k p) g -> p k g", p=128)))
        fw2t = wsm.tile([128, 2, E], MMDT, tag="ws")
        nc.sync.dma_start(out=fw2t,
                          in_=M(fw2.rearrange("(k p) e -> p k e", p=128)))
        rw1t = wsm.tile([128, KE, E4], MMDT, tag="ws")
        nc.sync.dma_start(out=rw1t,
                          in_=M(rw1.rearrange("(k p) g -> p k g", p=128)))
        rw2t = wrow.tile([128, E8], MMDT, tag="row")
        nc.sync.dma_start(out=rw2t, in_=M(rw2))
        rw3t = wrow.tile([E8, 16], MMDT, tag="row")
        v.memset(F(rw3t), 0.0)
        nc.sync.dma_start(out=rw3t[:, 0:c.OUT], in_=M(rw3))
        rb3_row = wrow.tile([1, 16], MMDT, tag="row")
        v.memset(F(rb3_row), 0.0)
        nc.sync.dma_start(out=rb3_row[:, 0:c.OUT], in_=M(rb3[None, :]))
        fb1_col = col_tile(fb1, 2, tag="fcol")
        fb2_col = col_tile(fb2, KE, tag="fcol")
        flng_col = col_tile(flng, KE, tag="fcol")
        flnb_col = col_tile(flnb, KE, tag="fcol")
        rb1_col = col_tile(rb1, 1, tag="fcol")
        rb2_col = wcol.tile([E8, 1], dt, tag="fcol")
        nc.sync.dma_start(out=rb2_col, in_=rb2[:, None])
        out_ap = out_dram.ap()

        bsrc, lsrc = rs["b", (L - 1, 3)], rs["l", (L - 1, 3)]
        for ci in range(NC):
            xb = load_x_chunk(bsrc, ci)
            xl = load_x_chunk(lsrc, ci)
            f1t = [pc.tile([128, C], MMDT, tag="a2", name=f"f1t{_i}")
                   for _i in range(2)]
            pfs = [ps.tile([128, C], F32, tag="mm", name=f"pf{_i}") for _i in range(2)]
            for k in range(2 * KE):
                rhs = xb[:, k, :] if k < KE else xl[:, k - KE, :]
                for m in range(2):
                    mm(pfs[m], fw1t[:, k, m * 128:(m + 1) * 128], rhs,
                       start=(k == 0), stop=(k == 2 * KE - 1))
            for m in range(2):
                f1f = pc.tile([128, C], dt, tag="a2")
                sc.activation(f1f, pfs[m], AF.Relu, bias=fb1_col[:, m:m + 1])
                v.tensor_scalar_min(f1t[m], f1f, 6.0)
            ft = pa.tile([128, KE, C], MMDT, tag="a8")
            pf2s = [ps.tile([128, C], F32, tag="mm", name=f"pf2{_i}") for _i in range(KE)]
            for k in range(2):
                for m in range(KE):
                    mm(pf2s[m], fw2t[:, k, m * 128:(m + 1) * 128],
                       f1t[k], start=(k == 0), stop=(k == 1))
            for m in range(KE):
                v.tensor_scalar_add(ft[:, m, :], pf2s[m], fb2_col[:, m:m + 1])
            frt = pa.tile([128, KE, C], MMDT, tag="a8")
            ln_stats_apply([ft[:, m, :] for m in range(KE)], flng_col,
                           flnb_col, [frt[:, m, :] for m in range(KE)],
                           relu=True, out_f32r=True)
            p1 = ps.tile([128, C], F32, tag="mm")
            for k in range(KE):
                mm(p1, rw1t[:, k, :], frt[:, k, :], start=(k == 0),
                   stop=(k == KE - 1))
            h1f = pc.tile([128, C], dt, tag="a2")
            sc.activation(h1f, p1, AF.Relu, bias=rb1_col[:, 0:1])
            h1t = pc.tile([128, C], MMDT, tag="a2")
            v.tensor_scalar_min(h1t, h1f, 6.0)
            p2 = ps.tile([E8, C], F32, tag="mm")
            mm(p2, rw2t, h1t, start=True, stop=True)
            h2f = pc.tile([E8, C], dt, tag="a2")
            sc.activation(h2f, p2, AF.Relu, bias=rb2_col[:, 0:1])
            h2t = pc.tile([E8, C], MMDT, tag="a2")
            v.tensor_scalar_min(h2t, h2f, 6.0)
            ot = pc.tile([128, NTT, c.OUT], dt, tag="a2")
            for tt in range(NTT):
                p3 = ps.tile([128, 16], F32, tag="mm")
                mm(p3, h2t[:, tt * 128:(tt + 1) * 128], rw3t,
                   start=True, stop=False)
                mm(p3, ONES_ROW[:, 0:128], rb3_row, start=False, stop=True)
                sc.activation(ot[:, tt, :], p3[:, 0:c.OUT], AF.Copy)
            nc.sync.dma_start(
                out=out_ap[ci * C:(ci + 1) * C, :].rearrange(
                    "(tt p) o -> p tt o", p=128),
                in_=ot)

    return din, out_dram


# ======================================================================
# kernel() entry point: full inputs in, full outputs out (8-core SPMD).
# ======================================================================
import concourse.bacc as _bacc
from concourse.bass_utils import run_bass_kernel_spmd as _run_spmd

_N_CORES = 8
_CACHE = {}


def _get_nc():
    if "nc" not in _CACHE:
        nc = _bacc.Bacc("TRN2", target_bir_lowering=False, debug=False)
        build(nc, Cfg())
        nc.finalize()
        _CACHE["nc"] = nc
    return _CACHE["nc"]


def kernel(**inputs):
    nc = _get_nc()
    cfg = Cfg()
    consts = host_constants(cfg)
    arr = {k: np.ascontiguousarray(np.asarray(v, dtype=np.float32))
           for k, v in inputs.items()}
    shared = {k: a for k, a in arr.items()
              if k not in ("body_feats", "limb_feats")}
    shared.update(consts)
    in_maps = []
    for i in range(_N_CORES):
        m = dict(shared)
        m["body_feats"] = np.ascontiguousarray(arr["body_feats"][i])
        m["limb_feats"] = np.ascontiguousarray(arr["limb_feats"][i])
        in_maps.append(m)
    res = run_kernel_spmd_cached(nc, in_maps)
    out = np.stack([res[i]["out"] for i in range(_N_CORES)], axis=0)
    return out.astype(np.float32)


def run_kernel_spmd_cached(nc, in_maps, **kw):
    r = _run_spmd(nc, in_maps, list(range(_N_CORES)), **kw)
    _CACHE["last_result"] = r
    return r.results



# revision 13
# speedup vs baseline: 1.4826x; 1.4826x over previous
"""Dual-stream linear-attention transformer kernel (per-core), v2.

Design vs v1 baseline:
  - fp16 matmul operands + activations + residual DRAM (fp32 PSUM/stats).
  - body/limb streams emitted interleaved (pair phases) so independent
    work keeps every engine queue fed (and PE out of low p-state).
  - q/k/v low-rank projections premultiplied on host to single [E,E]
    mats; k/v produced directly token-major by using x as lhsT.
  - depthwise conv runs on PE via host-built diagonal tap matrices
    (BN scale folded in) against halo-padded h tiles.
  - attention denominator scaled by 1/64 so fp16 reciprocals stay in
    the normal range (1/64 folded into the bd kv blocks).

Layouts:
  - layout 1: [E, N] feature-major; SBUF tiles [128, KE, C].
  - layout 2 (k/v only): [tok, E] token-major.
  - Residuals in internal DRAM as fp16 [E, N] -> p k n.
"""

from dataclasses import dataclass
from contextlib import ExitStack

import numpy as np

import concourse.bass as bass
import concourse.mybir as mybir
import concourse.tile as tile

F32 = mybir.dt.float32
F16 = mybir.dt.float16
AF = mybir.ActivationFunctionType
ALU = mybir.AluOpType

LN_EPS = 1e-5
BN_EPS = 1e-5
DEN_SCALE = 1.0 / 64.0


@dataclass
class Cfg:
    N: int = 2048
    E: int = 512
    R: int = 256
    X: int = 1024
    H: int = 8
    L: int = 3
    OUT: int = 15
    C: int = 512

    @property
    def KE(self):
        return self.E // 128

    @property
    def KX(self):
        return self.X // 128

    @property
    def NC(self):
        return self.N // self.C

    @property
    def NTT(self):
        return self.C // 128


def host_constants(cfg, inputs):
    """Precompute fp16 weights / fused constants on host."""
    c = cfg
    f = lambda a: np.ascontiguousarray(a, dtype=np.float32)
    h = lambda a: np.ascontiguousarray(a, dtype=np.float16)
    dw, uw = f(inputs["dw"]), f(inputs["uw"])
    # qkvw[l,a,t] = dw[l,a,t] @ uw[l,a,t]  [E,E]
    qkvw = np.einsum("latir,latrj->latij", dw, uw)
    out = {
        "body_feats": h(inputs["body_feats"]),
        "limb_feats": h(inputs["limb_feats"]),
        "qkvw": h(qkvw),
        "ub": f(inputs["ub"]),
        "ub16": h(inputs["ub"]),
        "ow": h(inputs["ow"]),
        "ob": f(inputs["ob"]),
        "w1": h(inputs["w1"]),
        "b1": f(inputs["b1"]),
        "w2": h(inputs["w2"]),
        "b2": f(inputs["b2"]),
        "lng": f(inputs["lng"]),
        "lnb": f(inputs["lnb"]),
        "gw1": h(inputs["gw1"]),
        "gb1": f(inputs["gb1"]),
        "gwd": h(f(inputs["gw2"])[:, :, 0] - f(inputs["gw2"])[:, :, 1]),
        "gb2d": f(f(inputs["gb2"])[:, 0:1] - f(inputs["gb2"])[:, 1:2]),
        "fw1": h(inputs["fw1"]),
        "fb1": f(inputs["fb1"]),
        "fw2": h(inputs["fw2"]),
        "fb2": f(inputs["fb2"]),
        "flng": f(inputs["flng"]),
        "flnb": f(inputs["flnb"]),
        "rw1": h(inputs["rw1"]),
        "rb1": f(inputs["rb1"]),
        "rw2": h(inputs["rw2"]),
        "rb2": f(inputs["rb2"]),
    }
    rw3 = np.zeros((c.E // 8, 16), np.float16)
    rw3[:, : c.OUT] = f(inputs["rw3"])
    out["rw3p"] = rw3
    rb3 = np.zeros((1, 16), np.float16)
    rb3[0, : c.OUT] = f(inputs["rb3"])
    out["rb3p"] = rb3
    # conv taps as diagonal matrices, BN scale folded in
    rsq = 1.0 / np.sqrt(1.0 + BN_EPS)
    A = f(inputs["bng"]) * rsq                        # [L,2,X]
    cw, cb = f(inputs["cw"]), f(inputs["cb"])
    taps = np.zeros((c.L, 2, c.KX, 3, 128, 128), np.float16)
    idx = np.arange(128)
    for t in range(3):
        wA = cw[:, :, :, t] * A                       # [L,2,X]
        wA = wA.reshape(c.L, 2, c.KX, 128)
        taps[:, :, :, t, idx, idx] = wA.astype(np.float16)
    out["taps"] = taps
    out["convB"] = f(cb * A + f(inputs["bnb"]))       # [L,2,X]
    out["ident"] = np.eye(128, dtype=np.float16)
    out["ones16"] = np.ones((128, 128), np.float16)
    E, H = c.E, c.H
    dh = E // H
    hmask = np.zeros((E, H), np.float16)
    for ff in range(E):
        hmask[ff, ff // dh] = 1.0
    out["hmask"] = hmask
    out["cmask"] = np.ascontiguousarray(hmask.T)
    return out


PHASES = []


def build(nc, cfg):
    c = cfg
    E, X, H, N, C, L = c.E, c.X, c.H, c.N, c.C, c.L
    KE, KX, NC, NTT = c.KE, c.KX, c.NC, c.NTT
    E4, E2, E8 = E // 4, E // 2, E // 8

    din = {}

    def inp(name, shape, dt):
        din[name] = nc.dram_tensor(name, list(shape), dt, kind="ExternalInput")
        return din[name].ap()

    body_feats = inp("body_feats", (N, E), F16)
    limb_feats = inp("limb_feats", (N, E), F16)
    qkvw = inp("qkvw", (L, 4, 3, E, E), F16)
    ub = inp("ub", (L, 4, 3, E), F32)
    ub16 = inp("ub16", (L, 4, 3, E), F16)
    ow = inp("ow", (L, 4, E, E), F16)
    ob = inp("ob", (L, 4, E), F32)
    w1 = inp("w1", (L, 2, E, X), F16)
    b1 = inp("b1", (L, 2, X), F32)
    w2 = inp("w2", (L, 2, X, E), F16)
    b2 = inp("b2", (L, 2, E), F32)
    taps = inp("taps", (L, 2, KX, 3, 128, 128), F16)
    convB = inp("convB", (L, 2, X), F32)
    lng = inp("lng", (L, 5, E), F32)
    lnb = inp("lnb", (L, 5, E), F32)
    gw1 = inp("gw1", (L, 2 * E, E4), F16)
    gb1 = inp("gb1", (L, E4), F32)
    gwd = inp("gwd", (L, E4), F16)
    gb2d = inp("gb2d", (L, 1), F32)
    fw1 = inp("fw1", (2 * E, E2), F16)
    fb1 = inp("fb1", (E2,), F32)
    fw2 = inp("fw2", (E2, E), F16)
    fb2 = inp("fb2", (E,), F32)
    flng = inp("flng", (E,), F32)
    flnb = inp("flnb", (E,), F32)
    rw1 = inp("rw1", (E, E4), F16)
    rb1 = inp("rb1", (E4,), F32)
    rw2 = inp("rw2", (E4, E8), F16)
    rb2 = inp("rb2", (E8,), F32)
    rw3p = inp("rw3p", (E8, 16), F16)
    rb3p = inp("rb3p", (1, 16), F16)
    ident_in = inp("ident", (128, 128), F16)
    ones_in = inp("ones16", (128, 128), F16)
    hmask_in = inp("hmask", (E, H), F16)
    cmask_in = inp("cmask", (H, E), F16)

    out_dram = nc.dram_tensor("out", [N, c.OUT], F32, kind="ExternalOutput")

    def idram(name):
        return nc.dram_tensor(name, [E, N], F16).ap().rearrange(
            "(k p) n -> p k n", p=128)

    rs = {}
    for s in ("b", "l"):
        rs[s, 0] = idram(f"r{s}0")
        for l in range(L):
            for st in (1, 2, 3):
                rs[s, (l, st)] = idram(f"r{s}_{l}_{st}")

    lowp = nc.allow_low_precision("fp16 activations within rel-err budget")

    with tile.TileContext(nc) as tc, ExitStack() as ctx, lowp:
        p_ = ctx.enter_context
        cst = p_(tc.tile_pool(name="cst", bufs=1))
        wq = p_(tc.tile_pool(name="wq", bufs=2))       # big weights
        wcol = p_(tc.tile_pool(name="wcol", bufs=20))  # bias cols
        wrow = p_(tc.tile_pool(name="wrow", bufs=2))   # bias rows
        pa = p_(tc.tile_pool(name="pa", bufs=9))       # 4KB fp16 act tiles
        pb = p_(tc.tile_pool(name="pb", bufs=4))       # ffn h tiles
        pc = p_(tc.tile_pool(name="pc", bufs=8))       # small tiles
        pat = p_(tc.tile_pool(name="pat", bufs=2))     # attn persistents
        ps = p_(tc.tile_pool(name="ps", bufs=4, space="PSUM"))
        pskv = p_(tc.tile_pool(name="pskv", bufs=2, space="PSUM"))
        psst = p_(tc.tile_pool(name="psst", bufs=2, space="PSUM"))

        v, sc, gp = nc.vector, nc.scalar, nc.gpsimd

        def mm(out, lhsT, rhs, start, stop):
            nc.tensor.matmul(out, lhsT, rhs, start=start, stop=stop)

        # ---- constants ----
        ident_t = cst.tile([128, 128], F16, tag="ident")
        nc.sync.dma_start(out=ident_t, in_=ident_in)
        ones_t = cst.tile([128, 128], F16, tag="ones")
        nc.sync.dma_start(out=ones_t, in_=ones_in)
        hmask_t = cst.tile([128, KE, H], F16, tag="hmask")
        nc.sync.dma_start(out=hmask_t,
                          in_=hmask_in.rearrange("(k p) h -> p k h", p=128))
        cmask_t = cst.tile([H, KE, 128], F16, tag="cmask")
        nc.sync.dma_start(out=cmask_t,
                          in_=cmask_in.rearrange("h (k p) -> h k p", p=128))
        ONES_COL = ones_t[:, 0:1]
        ONES_ROW = ones_t[0:1, :]
        eps_den = cst.tile([8, 1], F32, tag="epsd")
        v.memset(eps_den, 1e-6 * DEN_SCALE)
        eps_ln = cst.tile([1, 1], F32, tag="epsl")
        v.memset(eps_ln, LN_EPS)

        def col_tile(src_ap, m, tag="col"):
            t = wcol.tile([128, m], F32, tag=tag)
            nc.sync.dma_start(out=t, in_=src_ap.rearrange("(m p) -> p m", p=128))
            return t

        def ln_stats_apply(xs, g_col, b_col, outt, relu=False):
            """LayerNorm over features. xs: fp16 [128, KE, C]; outt same
            (outt doubles as x^2 scratch before the apply writes land)."""
            sq = outt
            v.tensor_tensor(out=sq, in0=xs, in1=xs, op=ALU.mult)
            ps_s = psst.tile([8, C], F32, tag="st")
            ps_ss = psst.tile([8, C], F32, tag="st")
            for m in range(KE):
                mm(ps_s[0:1, :], ONES_COL, xs[:, m, :], start=(m == 0),
                   stop=(m == KE - 1))
                mm(ps_ss[0:1, :], ONES_COL, sq[:, m, :], start=(m == 0),
                   stop=(m == KE - 1))
            arow = pc.tile([1, C], F32, tag="s2", bufs=4)   # mean
            brow = pc.tile([1, C], F32, tag="s2", bufs=4)   # msq -> var -> sd
            trow2 = pc.tile([1, C], F32, tag="s2", bufs=4)  # mean^2
            sc.activation(arow, ps_s[0:1, :], AF.Copy, scale=1.0 / E)
            sc.activation(brow, ps_ss[0:1, :], AF.Copy, scale=1.0 / E)
            sc.activation(trow2, arow, AF.Square)
            v.tensor_tensor(out=brow, in0=brow, in1=trow2, op=ALU.subtract)
            sc.activation(brow, brow, AF.Sqrt, bias=eps_ln[0:1, 0:1])
            stt = pc.tile([1, 2, C], F16, tag="s2", bufs=4)
            v.reciprocal(out=stt[:, 0, :], in_=brow)
            v.tensor_tensor(out=stt[:, 1, :], in0=arow, in1=stt[:, 0, :],
                            op=ALU.mult)
            bc_s = ps.tile([128, C], F32, tag="mm")
            mm(bc_s, ones_t[0:1, :], stt[0:1, 0, :], start=True, stop=True)
            bc_t = ps.tile([128, C], F32, tag="mm")
            mm(bc_t, ones_t[0:1, :], stt[0:1, 1, :], start=True, stop=True)
            sb = pc.tile([128, 2, C], F16, tag="c2", bufs=3)
            sc.activation(sb[:, 0, :], bc_s, AF.Copy)
            sc.activation(sb[:, 1, :], bc_t, AF.Copy)
            fn = AF.Relu if relu else AF.Identity
            for m in range(KE):
                t1 = pc.tile([128, C], F16, tag="c1")
                v.tensor_tensor(out=t1, in0=xs[:, m, :], in1=sb[:, 0, :],
                                op=ALU.mult)
                v.tensor_tensor(out=t1, in0=t1, in1=sb[:, 1, :],
                                op=ALU.subtract)
                sc.activation(outt[:, m, :], t1, fn,
                              bias=b_col[:, m:m + 1], scale=g_col[:, m:m + 1])

        def load_x_chunk(dram_l1, ci, tag="a4"):
            xt = pa.tile([128, KE, C], F16, tag=tag)
            nc.sync.dma_start(out=xt, in_=dram_l1[:, :, ci * C:(ci + 1) * C])
            return xt

        def store_chunk(dram_l1, ci, t):
            gp.dma_start(out=dram_l1[:, :, ci * C:(ci + 1) * C], in_=t)

        # ---- entry transpose (interleaved) ----
        def entry_tile(x_ap, dst, ttk):
            x2 = pa.tile([128, E], F16, tag="a4")
            nc.sync.dma_start(out=x2, in_=x_ap[ttk * 128:(ttk + 1) * 128, :])
            xt = pa.tile([128, KE, 128], F16, tag="a4")
            for f in range(KE):
                pt = ps.tile([128, 128], F16, tag="mm")
                nc.tensor.transpose(pt, x2[:, f * 128:(f + 1) * 128], ident_t)
                sc.activation(xt[:, f, :], pt, AF.Copy)
            nc.sync.dma_start(out=dst[:, :, ttk * 128:(ttk + 1) * 128], in_=xt)

        PHASES.append(("entry", len(nc.inst_map)))
        for ttk in range(N // 128):
            entry_tile(body_feats, rs["b", 0], ttk)
            entry_tile(limb_feats, rs["l", 0], ttk)

        # ---- linear attention (pairs) ----
        def attn_setup(l, a):
            st = {}
            qt = wq.tile([128, KE, 3, E], F16, tag="qkv")
            for t3 in range(3):
                nc.sync.dma_start(
                    out=qt[:, :, t3, :],
                    in_=qkvw[l, a, t3].rearrange("(k p) e -> p k e", p=128))
            owt = wq.tile([128, KE, E], F16, tag="ow")
            nc.sync.dma_start(
                out=owt, in_=ow[l, a].rearrange("(k p) e -> p k e", p=128))
            st["qt"], st["owt"] = qt, owt
            st["ubq_col"] = col_tile(ub[l, a, 0], KE)
            ubkv = wrow.tile([1, 2, E], F16, tag="row")
            nc.sync.dma_start(out=ubkv[:, 0, :], in_=ub16[l, a, 1][None, :])
            nc.sync.dma_start(out=ubkv[:, 1, :], in_=ub16[l, a, 2][None, :])
            st["ubkv"] = ubkv
            st["ob_col"] = col_tile(ob[l, a], KE)
            st["kv_acc"] = pat.tile([128, 4, 258], F32, tag="kva",
                                    name="kv_acc")
            return st

        def alpha_step(st, xkv_dram, ci):
            qt = st["qt"]
            xt = load_x_chunk(xkv_dram, ci)
            k2f = pa.tile([128, NTT, E], F16, tag="a4")
            v2x = pa.tile([128, NTT, 2, 258], F16, tag="a4")
            v.memset(v2x[:, :, :, 256:258], 1.0)
            for tt in range(NTT):
                pk = ps.tile([128, E], F32, tag="mm")
                pv = ps.tile([128, E], F32, tag="mm")
                for k in range(KE):
                    lx = xt[:, k, tt * 128:(tt + 1) * 128]
                    mm(pk, lx, qt[:, k, 1, :], start=(k == 0), stop=False)
                    mm(pv, lx, qt[:, k, 2, :], start=(k == 0), stop=False)
                mm(pk, ONES_ROW, st["ubkv"][:, 0, :], start=False, stop=True)
                mm(pv, ONES_ROW, st["ubkv"][:, 1, :], start=False, stop=True)
                ee = pc.tile([128, E], F16, tag="c1")
                rr = pc.tile([128, E], F16, tag="c1")
                sc.activation(ee, pk, AF.Exp)
                v.tensor_scalar_max(rr, pk, 0.0)
                v.tensor_scalar_min(ee, ee, 1.0)
                v.tensor_tensor(out=k2f[:, tt, :], in0=ee, in1=rr, op=ALU.add)
                sc.activation(v2x[:, tt, 0, 0:256], pv[:, 0:256], AF.Copy)
                sc.activation(v2x[:, tt, 1, 0:256], pv[:, 256:512], AF.Copy)
            kv_acc = st["kv_acc"]
            for p in range(4):
                pkv = pskv.tile([128, 258], F32, tag="kv")
                for tt in range(NTT):
                    mm(pkv, k2f[:, tt, p * 128:(p + 1) * 128],
                       v2x[:, tt, p // 2, :],
                       start=(tt == 0), stop=(tt == NTT - 1))
                if ci == 0:
                    sc.activation(kv_acc[:, p, :], pkv, AF.Copy)
                else:
                    v.tensor_tensor(out=kv_acc[:, p, :], in0=kv_acc[:, p, :],
                                    in1=pkv, op=ALU.add)

        def alpha_fin(st):
            kv_acc = st["kv_acc"]
            bd = pat.tile([128, KE, 128], F16, tag="bd")
            v.memset(bd, 0.0)
            for p in range(4):
                h0c = (2 * p % 4) * 64
                h1c = ((2 * p + 1) % 4) * 64
                v.tensor_scalar_mul(bd[0:64, p, 0:64],
                                    kv_acc[0:64, p, h0c:h0c + 64], DEN_SCALE)
                v.tensor_scalar_mul(bd[64:128, p, 64:128],
                                    kv_acc[64:128, p, h1c:h1c + 64], DEN_SCALE)
            kmm = pat.tile([128, KE, H], F16, tag="km")
            for k in range(KE):
                v.tensor_scalar_mul(kmm[:, k, :], hmask_t[:, k, :],
                                    kv_acc[:, k, 256:257])
            st["bd"], st["kmm"] = bd, kmm

        def beta_step(st, xq_dram, ci, tail):
            qt = st["qt"]
            xq = load_x_chunk(xq_dram, ci)
            qf = pa.tile([128, KE, C], F16, tag="a4")
            for m in range(KE):
                pq = ps.tile([128, C], F32, tag="mm")
                for k in range(KE):
                    mm(pq, qt[:, k, 0, m * 128:(m + 1) * 128], xq[:, k, :],
                       start=(k == 0), stop=(k == KE - 1))
                ee = pc.tile([128, C], F16, tag="c1")
                rr = pc.tile([128, C], F16, tag="c1")
                sc.activation(ee, pq, AF.Exp,
                              bias=st["ubq_col"][:, m:m + 1])
                v.tensor_scalar(out=rr, in0=pq,
                                scalar1=st["ubq_col"][:, m:m + 1],
                                scalar2=0.0, op0=ALU.add, op1=ALU.max)
                v.tensor_scalar_min(ee, ee, 1.0)
                v.tensor_tensor(out=qf[:, m, :], in0=ee, in1=rr, op=ALU.add)
            pd = psst.tile([8, C], F32, tag="st")
            for k in range(KE):
                mm(pd, st["kmm"][:, k, :], qf[:, k, :], start=(k == 0),
                   stop=(k == KE - 1))
            den = pc.tile([8, C], F32, tag="s2", bufs=4)
            sc.activation(den, pd, AF.Identity, bias=eps_den,
                          scale=DEN_SCALE)
            rec = pc.tile([8, C], F16, tag="s2", bufs=4)
            v.reciprocal(out=rec, in_=den)
            att = pa.tile([128, KE, C], F16, tag="a4")
            for m in range(KE):
                pn = ps.tile([128, C], F32, tag="mm")
                mm(pn, st["bd"][:, m, :], qf[:, m, :], start=True, stop=True)
                pr = ps.tile([128, C], F32, tag="mm")
                mm(pr, cmask_t[:, m, :], rec, start=True, stop=True)
                rb = pc.tile([128, C], F16, tag="c1")
                sc.activation(rb, pr, AF.Copy)
                v.tensor_tensor(out=att[:, m, :], in0=pn, in1=rb, op=ALU.mult)
            proj = pa.tile([128, KE, C], F16, tag="a4")
            for m in range(KE):
                po = ps.tile([128, C], F32, tag="mm")
                for k in range(KE):
                    mm(po, st["owt"][:, k, m * 128:(m + 1) * 128],
                       att[:, k, :], start=(k == 0), stop=(k == KE - 1))
                sc.activation(proj[:, m, :], po, AF.Identity,
                              bias=st["ob_col"][:, m:m + 1])
            tail(ci, proj, xq)

        # ---- tails ----
        def make_self_tail(l, s, dst):
            g_col = col_tile(lng[l, 0 if s == "b" else 1], KE, tag="lncol")
            b_col = col_tile(lnb[l, 0 if s == "b" else 1], KE, tag="lncol")

            def tail(ci, proj, xq):
                v.tensor_tensor(out=proj, in0=proj, in1=xq, op=ALU.add)
                outt = pa.tile([128, KE, C], F16, tag="a4")
                ln_stats_apply(proj, g_col, b_col, outt)
                store_chunk(dst, ci, outt)

            return tail

        def make_cross_tail(l, s, dst):
            gw1t = wq.tile([128, 2 * KE, E4], F16, tag="gw")
            nc.sync.dma_start(out=gw1t,
                              in_=gw1[l].rearrange("(k p) g -> p k g", p=128))
            gwd_col = wcol.tile([128, 1], F16, tag="gwd")
            nc.sync.dma_start(out=gwd_col,
                              in_=gwd[l].rearrange("(m p) -> p m", p=128))
            gb1_col = col_tile(gb1[l], 1, tag="lncol")
            gb2d_t = wcol.tile([1, 1], F32, tag="gb2d")
            nc.sync.dma_start(out=gb2d_t, in_=gb2d[l][None, :])
            g_col = col_tile(lng[l, 2], KE, tag="lncol")
            b_col = col_tile(lnb[l, 2], KE, tag="lncol")

            def tail(ci, proj, xq):
                pg = ps.tile([128, C], F32, tag="mm")
                for k in range(2 * KE):
                    rhs = xq[:, k, :] if k < KE else proj[:, k - KE, :]
                    mm(pg, gw1t[:, k, :], rhs, start=(k == 0),
                       stop=(k == 2 * KE - 1))
                g1f = pc.tile([128, C], F16, tag="c1")
                sc.activation(g1f, pg, AF.Relu, bias=gb1_col[:, 0:1])
                g1t = pc.tile([128, C], F16, tag="c1")
                v.tensor_scalar_min(g1t, g1f, 6.0)
                pg2 = psst.tile([8, C], F32, tag="st")
                mm(pg2[0:1, :], gwd_col, g1t, start=True, stop=True)
                bgf = pc.tile([1, C], F16, tag="s2", bufs=4)
                sc.activation(bgf, pg2[0:1, :], AF.Sigmoid,
                              bias=gb2d_t[0:1, 0:1])
                pbg = ps.tile([128, C], F32, tag="mm")
                mm(pbg, ones_t[0:1, :], bgf, start=True, stop=True)
                bgt = pc.tile([128, C], F16, tag="c1")
                sc.activation(bgt, pbg, AF.Copy)
                mt = pa.tile([128, KE, C], F16, tag="a4")
                v.tensor_tensor(out=mt, in0=xq, in1=proj, op=ALU.subtract)
                for m in range(KE):
                    v.tensor_tensor(out=mt[:, m, :], in0=mt[:, m, :],
                                    in1=bgt, op=ALU.mult)
                v.tensor_tensor(out=mt, in0=mt, in1=proj, op=ALU.add)
                outt = pa.tile([128, KE, C], F16, tag="a4")
                ln_stats_apply(mt, g_col, b_col, outt)
                store_chunk(dst, ci, outt)

            return tail

        # ---- FFN pair ----
        def ffn_setup(l, s):
            si = 0 if s == "b" else 1
            st = {}
            w1t = wq.tile([128, KE, X], F16, tag="w1")
            nc.sync.dma_start(
                out=w1t, in_=w1[l, si].rearrange("(k p) x -> p k x", p=128))
            w2t = wq.tile([128, KX, E], F16, tag="w2")
            nc.sync.dma_start(
                out=w2t, in_=w2[l, si].rearrange("(k p) e -> p k e", p=128))
            tapt = wq.tile([128, KX, 3, 128], F16, tag="tp" + s, bufs=1)
            nc.sync.dma_start(out=tapt,
                              in_=taps[l, si].rearrange("m t p f -> p m t f"))
            st["w1t"], st["w2t"], st["tapt"] = w1t, w2t, tapt
            st["b1_col"] = col_tile(b1[l, si], KX, tag="ffcol")
            st["b2_col"] = col_tile(b2[l, si], KE, tag="ffcol")
            st["B_col"] = col_tile(convB[l, si], KX, tag="ffcol")
            st["g_col"] = col_tile(lng[l, 3 if s == "b" else 4], KE,
                                   tag="lncol")
            st["bb_col"] = col_tile(lnb[l, 3 if s == "b" else 4], KE,
                                    tag="lncol")
            st["hts"] = [None] * NC
            st["xts"] = [None] * NC
            return st

        def ffn_h(st, src, ci):
            xt = load_x_chunk(src, ci)
            st["xts"][ci] = xt
            ht = pb.tile([128, KX, C + 2], F16, tag="ht")
            if ci == 0:
                v.memset(ht[:, :, 0:1], 0.0)
            for m in range(KX):
                ph = ps.tile([128, C], F32, tag="mm")
                for k in range(KE):
                    mm(ph, st["w1t"][:, k, m * 128:(m + 1) * 128],
                       xt[:, k, :], start=(k == 0), stop=(k == KE - 1))
                sc.activation(ht[:, m, 1:C + 1], ph, AF.Relu,
                              bias=st["b1_col"][:, m:m + 1])
                v.tensor_scalar_min(ht[:, m, 1:C + 1], ht[:, m, 1:C + 1],
                                    6.0)
            prev = st["hts"][ci - 1] if ci > 0 else None
            if prev is not None:
                v.tensor_copy(prev[:, :, C + 1:C + 2], ht[:, :, 1:2])
                v.tensor_copy(ht[:, :, 0:1], prev[:, :, C:C + 1])
            if ci == NC - 1:
                v.memset(ht[:, :, C + 1:C + 2], 0.0)
            st["hts"][ci] = ht

        def ffn_tail(st, dst, ci):
            ht = st["hts"][ci]
            h2 = pb.tile([128, KX, C], F16, tag="h2", bufs=2)
            for m in range(KX):
                pacc = ps.tile([128, C], F32, tag="mm")
                for t in range(3):
                    mm(pacc, st["tapt"][:, m, t, :], ht[:, m, t:t + C],
                       start=(t == 0), stop=(t == 2))
                rel = pc.tile([128, C], F16, tag="c1")
                sc.activation(rel, pacc, AF.Relu, bias=st["B_col"][:, m:m + 1])
                v.tensor_scalar_min(h2[:, m, :], rel, 6.0)
            rt = pa.tile([128, KE, C], F16, tag="a4")
            for m in range(KE):
                pw = ps.tile([128, C], F32, tag="mm")
                for k in range(KX):
                    mm(pw, st["w2t"][:, k, m * 128:(m + 1) * 128],
                       h2[:, k, :], start=(k == 0), stop=(k == KX - 1))
                sc.activation(rt[:, m, :], pw, AF.Identity,
                              bias=st["b2_col"][:, m:m + 1])
            v.tensor_tensor(out=rt, in0=rt, in1=st["xts"][ci], op=ALU.add)
            outt = pa.tile([128, KE, C], F16, tag="a4")
            ln_stats_apply(rt, st["g_col"], st["bb_col"], outt)
            store_chunk(dst, ci, outt)
            st["hts"][ci] = st["xts"][ci] = None

        # ---- layers ----
        for l in range(L):
            bsrc = rs["b", 0] if l == 0 else rs["b", (l - 1, 3)]
            lsrc = rs["l", 0] if l == 0 else rs["l", (l - 1, 3)]

            PHASES.append((f"attnA{l}.alpha", len(nc.inst_map)))
            st0 = attn_setup(l, 0)
            st1 = attn_setup(l, 1)
            for ci in range(NC):
                alpha_step(st0, bsrc, ci)
                alpha_step(st1, lsrc, ci)
            alpha_fin(st0)
            alpha_fin(st1)
            PHASES.append((f"attnA{l}.beta", len(nc.inst_map)))
            t0 = make_self_tail(l, "b", rs["b", (l, 1)])
            t1 = make_self_tail(l, "l", rs["l", (l, 1)])
            for ci in range(NC):
                beta_step(st0, bsrc, ci, t0)
                beta_step(st1, lsrc, ci, t1)

            PHASES.append((f"attnB{l}.alpha", len(nc.inst_map)))
            b1d, l1d = rs["b", (l, 1)], rs["l", (l, 1)]
            st2 = attn_setup(l, 2)
            st3 = attn_setup(l, 3)
            for ci in range(NC):
                alpha_step(st2, l1d, ci)
                alpha_step(st3, b1d, ci)
            alpha_fin(st2)
            alpha_fin(st3)
            PHASES.append((f"attnB{l}.beta", len(nc.inst_map)))
            t2 = make_cross_tail(l, "b", rs["b", (l, 2)])
            t3 = make_cross_tail(l, "l", rs["l", (l, 2)])
            for ci in range(NC):
                beta_step(st2, b1d, ci, t2)
                beta_step(st3, l1d, ci, t3)

            PHASES.append((f"ffn{l}", len(nc.inst_map)))
            fb_ = ffn_setup(l, "b")
            fl_ = ffn_setup(l, "l")
            b2d, l2d = rs["b", (l, 2)], rs["l", (l, 2)]
            ffn_h(fb_, b2d, 0)
            ffn_h(fl_, l2d, 0)
            for ci in range(1, NC):
                ffn_h(fb_, b2d, ci)
                ffn_tail(fb_, rs["b", (l, 3)], ci - 1)
                ffn_h(fl_, l2d, ci)
                ffn_tail(fl_, rs["l", (l, 3)], ci - 1)
            ffn_tail(fb_, rs["b", (l, 3)], NC - 1)
            ffn_tail(fl_, rs["l", (l, 3)], NC - 1)

        PHASES.append(("final", len(nc.inst_map)))
        # ---- final head ----
        fw1t = wq.tile([128, 2 * KE, E2], F16, tag="w1")
        nc.sync.dma_start(out=fw1t,
                          in_=fw1.rearrange("(k p) g -> p k g", p=128))
        fw2t = wq.tile([128, 2, E], F16, tag="gw")
        nc.sync.dma_start(out=fw2t,
                          in_=fw2.rearrange("(k p) e -> p k e", p=128))
        rw1t = wq.tile([128, KE, E4], F16, tag="gw")
        nc.sync.dma_start(out=rw1t,
                          in_=rw1.rearrange("(k p) g -> p k g", p=128))
        rw2t = wrow.tile([128, E8], F16, tag="row2")
        nc.sync.dma_start(out=rw2t, in_=rw2)
        rw3t = wrow.tile([E8, 16], F16, tag="row2")
        nc.sync.dma_start(out=rw3t, in_=rw3p)
        rb3_row = wrow.tile([1, 16], F16, tag="row")
        nc.sync.dma_start(out=rb3_row, in_=rb3p)
        fb1_col = col_tile(fb1, 2, tag="fcol")
        fb2_col = col_tile(fb2, KE, tag="fcol")
        flng_col = col_tile(flng, KE, tag="fcol")
        flnb_col = col_tile(flnb, KE, tag="fcol")
        rb1_col = col_tile(rb1, 1, tag="fcol")
        rb2_col = wcol.tile([E8, 1], F32, tag="fcol")
        nc.sync.dma_start(out=rb2_col, in_=rb2[:, None])
        out_ap = out_dram.ap()

        bsrc, lsrc = rs["b", (L - 1, 3)], rs["l", (L - 1, 3)]
        for ci in range(NC):
            xb = load_x_chunk(bsrc, ci)
            xl = load_x_chunk(lsrc, ci)
            f1t = [pc.tile([128, C], F16, tag="c1", name=f"f1t{_i}")
                   for _i in range(2)]
            for m in range(2):
                pf = ps.tile([128, C], F32, tag="mm")
                for k in range(2 * KE):
                    rhs = xb[:, k, :] if k < KE else xl[:, k - KE, :]
                    mm(pf, fw1t[:, k, m * 128:(m + 1) * 128], rhs,
                       start=(k == 0), stop=(k == 2 * KE - 1))
                f1f = pc.tile([128, C], F16, tag="c1")
                sc.activation(f1f, pf, AF.Relu, bias=fb1_col[:, m:m + 1])
                v.tensor_scalar_min(f1t[m], f1f, 6.0)
            ft = pa.tile([128, KE, C], F16, tag="a4")
            for m in range(KE):
                pf2 = ps.tile([128, C], F32, tag="mm")
                for k in range(2):
                    mm(pf2, fw2t[:, k, m * 128:(m + 1) * 128],
                       f1t[k], start=(k == 0), stop=(k == 1))
                sc.activation(ft[:, m, :], pf2, AF.Identity,
                              bias=fb2_col[:, m:m + 1])
            frt = pa.tile([128, KE, C], F16, tag="a4")
            ln_stats_apply(ft, flng_col, flnb_col, frt, relu=True)
            p1 = ps.tile([128, C], F32, tag="mm")
            for k in range(KE):
                mm(p1, rw1t[:, k, :], frt[:, k, :], start=(k == 0),
                   stop=(k == KE - 1))
            h1f = pc.tile([128, C], F16, tag="c1")
            sc.activation(h1f, p1, AF.Relu, bias=rb1_col[:, 0:1])
            h1t = pc.tile([128, C], F16, tag="c1")
            v.tensor_scalar_min(h1t, h1f, 6.0)
            p2 = ps.tile([E8, C], F32, tag="mm")
            mm(p2, rw2t, h1t, start=True, stop=True)
            h2f = pc.tile([E8, C], F16, tag="c1")
            sc.activation(h2f, p2, AF.Relu, bias=rb2_col[:, 0:1])
            h2t = pc.tile([E8, C], F16, tag="c1")
            v.tensor_scalar_min(h2t, h2f, 6.0)
            ot = pc.tile([128, NTT, c.OUT], F32, tag="c2", bufs=3)
            for tt in range(NTT):
                p3 = ps.tile([128, 16], F32, tag="mm")
                mm(p3, h2t[:, tt * 128:(tt + 1) * 128], rw3t,
                   start=True, stop=False)
                mm(p3, ONES_ROW[:, 0:128], rb3_row, start=False, stop=True)
                sc.activation(ot[:, tt, :], p3[:, 0:c.OUT], AF.Copy)
            nc.sync.dma_start(
                out=out_ap[ci * C:(ci + 1) * C, :].rearrange(
                    "(tt p) o -> p tt o", p=128),
                in_=ot)

    return din, out_dram


# ======================================================================
# kernel() entry point: full inputs in, full outputs out (8-core SPMD).
# ======================================================================
import concourse.bacc as _bacc
from concourse.bass_utils import run_bass_kernel_spmd as _run_spmd

_N_CORES = 8
_CACHE = {}


def _get_nc():
    if "nc" not in _CACHE:
        nc = _bacc.Bacc("TRN2", target_bir_lowering=False, debug=False)
        build(nc, Cfg())
        nc.finalize()
        _CACHE["nc"] = nc
    return _CACHE["nc"]


def kernel(**inputs):
    nc = _get_nc()
    cfg = Cfg()
    arr = {k: np.asarray(v) for k, v in inputs.items()}
    consts = host_constants(cfg, arr)
    shared = {k: a for k, a in consts.items()
              if k not in ("body_feats", "limb_feats")}
    in_maps = []
    for i in range(_N_CORES):
        m = dict(shared)
        m["body_feats"] = np.ascontiguousarray(consts["body_feats"][i])
        m["limb_feats"] = np.ascontiguousarray(consts["limb_feats"][i])
        in_maps.append(m)
    res = run_kernel_spmd_cached(nc, in_maps)
    out = np.stack([res[i]["out"] for i in range(_N_CORES)], axis=0)
    return out.astype(np.float32)


def run_kernel_spmd_cached(nc, in_maps, **kw):
    r = _run_spmd(nc, in_maps, list(range(_N_CORES)), **kw)
    _CACHE["last_result"] = r
    return r.results


# revision 20
# speedup vs baseline: 1.7321x; 1.1683x over previous
"""Dual-stream linear-attention transformer kernel (per-core), v3.

v3: per-m interleaved emission across the body/limb pair (ACT has no
exec-queue lookahead, so fine-grained alternation is what keeps it fed),
LN stats pairs share one PSUM tile (rows 0/1), fp16 operands throughout.
See v2 notes below.

  - fp16 matmul operands + activations + residual DRAM (fp32 PSUM/stats).
  - q/k/v low-rank projections premultiplied on host to single [E,E]
    mats; k/v produced directly token-major by using x as lhsT.
  - depthwise conv on PE via host-built diagonal tap matrices (BN scale
    folded in) against halo-padded h tiles.
  - attention denominator scaled by 1/64 so fp16 reciprocals stay in
    the normal range (1/64 folded into the bd kv blocks).

Layouts:
  - layout 1: [E, N] feature-major; SBUF tiles [128, KE, C].
  - layout 2 (k/v only): [tok, E] token-major.
  - Residuals in internal DRAM as fp16 [E, N] -> p k n.
"""

from dataclasses import dataclass
from contextlib import ExitStack

import numpy as np

import concourse.bass as bass
import concourse.mybir as mybir
import concourse.tile as tile

F32 = mybir.dt.float32
F16 = mybir.dt.float16
AF = mybir.ActivationFunctionType
ALU = mybir.AluOpType

LN_EPS = 1e-5
BN_EPS = 1e-5
DEN_SCALE = 1.0 / 64.0


@dataclass
class Cfg:
    N: int = 2048
    E: int = 512
    R: int = 256
    X: int = 1024
    H: int = 8
    L: int = 3
    OUT: int = 15
    C: int = 512

    @property
    def KE(self):
        return self.E // 128

    @property
    def KX(self):
        return self.X // 128

    @property
    def NC(self):
        return self.N // self.C

    @property
    def NTT(self):
        return self.C // 128


def host_constants(cfg, inputs):
    """Precompute fp16 weights / fused constants on host."""
    c = cfg
    f = lambda a: np.ascontiguousarray(a, dtype=np.float32)
    h = lambda a: np.ascontiguousarray(a, dtype=np.float16)
    dw, uw = f(inputs["dw"]), f(inputs["uw"])
    qkvw = np.einsum("latir,latrj->latij", dw, uw)
    out = {
        "body_feats": h(inputs["body_feats"]),
        "limb_feats": h(inputs["limb_feats"]),
        "qkvw": h(qkvw),
        "ub": f(inputs["ub"]),
        "ub16": h(inputs["ub"]),
        "ow": h(inputs["ow"]),
        "ob": f(inputs["ob"]),
        "w1": h(inputs["w1"]),
        "b1": f(inputs["b1"]),
        "w2": h(inputs["w2"]),
        "b2": f(inputs["b2"]),
        "lng": f(inputs["lng"]),
        "lnb": f(inputs["lnb"]),
        "gw1": h(inputs["gw1"]),
        "gb1": f(inputs["gb1"]),
        "gwd": h(f(inputs["gw2"])[:, :, 0] - f(inputs["gw2"])[:, :, 1]),
        "gb2d": f(f(inputs["gb2"])[:, 0:1] - f(inputs["gb2"])[:, 1:2]),
        "fw1": h(inputs["fw1"]),
        "fb1": f(inputs["fb1"]),
        "fw2": h(inputs["fw2"]),
        "fb2": f(inputs["fb2"]),
        "flng": f(inputs["flng"]),
        "flnb": f(inputs["flnb"]),
        "rw1": h(inputs["rw1"]),
        "rb1": f(inputs["rb1"]),
        "rw2": h(inputs["rw2"]),
        "rb2": f(inputs["rb2"]),
    }
    rw3 = np.zeros((c.E // 8, 16), np.float16)
    rw3[:, : c.OUT] = f(inputs["rw3"])
    out["rw3p"] = rw3
    rb3 = np.zeros((1, 16), np.float16)
    rb3[0, : c.OUT] = f(inputs["rb3"])
    out["rb3p"] = rb3
    rsq = 1.0 / np.sqrt(1.0 + BN_EPS)
    A = f(inputs["bng"]) * rsq
    cw, cb = f(inputs["cw"]), f(inputs["cb"])
    taps = np.zeros((c.L, 2, c.KX, 3, 128, 128), np.float16)
    idx = np.arange(128)
    for t in range(3):
        wA = (cw[:, :, :, t] * A).reshape(c.L, 2, c.KX, 128)
        taps[:, :, :, t, idx, idx] = wA.astype(np.float16)
    out["taps"] = taps
    out["convB"] = f(cb * A + f(inputs["bnb"]))
    out["ident"] = np.eye(128, dtype=np.float16)
    out["ones16"] = np.ones((128, 128), np.float16)
    E, H = c.E, c.H
    dh = E // H
    hmask = np.zeros((E, H), np.float16)
    for ff in range(E):
        hmask[ff, ff // dh] = 1.0
    out["hmask"] = hmask
    out["cmask"] = np.ascontiguousarray(hmask.T)
    return out


PHASES = []


def build(nc, cfg):
    c = cfg
    E, X, H, N, C, L = c.E, c.X, c.H, c.N, c.C, c.L
    KE, KX, NC, NTT = c.KE, c.KX, c.NC, c.NTT
    E4, E2, E8 = E // 4, E // 2, E // 8

    din = {}

    def inp(name, shape, dt):
        din[name] = nc.dram_tensor(name, list(shape), dt, kind="ExternalInput")
        return din[name].ap()

    body_feats = inp("body_feats", (N, E), F16)
    limb_feats = inp("limb_feats", (N, E), F16)
    qkvw = inp("qkvw", (L, 4, 3, E, E), F16)
    ub = inp("ub", (L, 4, 3, E), F32)
    ub16 = inp("ub16", (L, 4, 3, E), F16)
    ow = inp("ow", (L, 4, E, E), F16)
    ob = inp("ob", (L, 4, E), F32)
    w1 = inp("w1", (L, 2, E, X), F16)
    b1 = inp("b1", (L, 2, X), F32)
    w2 = inp("w2", (L, 2, X, E), F16)
    b2 = inp("b2", (L, 2, E), F32)
    taps = inp("taps", (L, 2, KX, 3, 128, 128), F16)
    convB = inp("convB", (L, 2, X), F32)
    lng = inp("lng", (L, 5, E), F32)
    lnb = inp("lnb", (L, 5, E), F32)
    gw1 = inp("gw1", (L, 2 * E, E4), F16)
    gb1 = inp("gb1", (L, E4), F32)
    gwd = inp("gwd", (L, E4), F16)
    gb2d = inp("gb2d", (L, 1), F32)
    fw1 = inp("fw1", (2 * E, E2), F16)
    fb1 = inp("fb1", (E2,), F32)
    fw2 = inp("fw2", (E2, E), F16)
    fb2 = inp("fb2", (E,), F32)
    flng = inp("flng", (E,), F32)
    flnb = inp("flnb", (E,), F32)
    rw1 = inp("rw1", (E, E4), F16)
    rb1 = inp("rb1", (E4,), F32)
    rw2 = inp("rw2", (E4, E8), F16)
    rb2 = inp("rb2", (E8,), F32)
    rw3p = inp("rw3p", (E8, 16), F16)
    rb3p = inp("rb3p", (1, 16), F16)
    ident_in = inp("ident", (128, 128), F16)
    ones_in = inp("ones16", (128, 128), F16)
    hmask_in = inp("hmask", (E, H), F16)
    cmask_in = inp("cmask", (H, E), F16)

    out_dram = nc.dram_tensor("out", [N, c.OUT], F32, kind="ExternalOutput")

    def idram(name):
        return nc.dram_tensor(name, [E, N], F16).ap().rearrange(
            "(k p) n -> p k n", p=128)

    rs = {}
    for s in ("b", "l"):
        rs[s, 0] = idram(f"r{s}0")
        for l in range(L):
            for st in (1, 2, 3):
                rs[s, (l, st)] = idram(f"r{s}_{l}_{st}")

    lowp = nc.allow_low_precision("fp16 activations within rel-err budget")

    with tile.TileContext(nc) as tc, ExitStack() as ctx, lowp:
        p_ = ctx.enter_context
        cst = p_(tc.tile_pool(name="cst", bufs=1))
        wq = p_(tc.tile_pool(name="wq", bufs=2))       # big weights
        wcol = p_(tc.tile_pool(name="wcol", bufs=20))  # bias cols
        wrow = p_(tc.tile_pool(name="wrow", bufs=2))   # bias rows
        pa = p_(tc.tile_pool(name="pa", bufs=8))       # 4KB fp16 act tiles
        pb = p_(tc.tile_pool(name="pb", bufs=4))       # ffn h tiles
        pc = p_(tc.tile_pool(name="pc", bufs=6))       # small tiles
        pat = p_(tc.tile_pool(name="pat", bufs=2))     # attn persistents
        ps = p_(tc.tile_pool(name="ps", bufs=4, space="PSUM"))
        pskv = p_(tc.tile_pool(name="pskv", bufs=2, space="PSUM"))
        psst = p_(tc.tile_pool(name="psst", bufs=2, space="PSUM"))

        v, sc, gp = nc.vector, nc.scalar, nc.gpsimd

        def mm(out, lhsT, rhs, start, stop):
            nc.tensor.matmul(out, lhsT, rhs, start=start, stop=stop)

        # ---- constants ----
        ident_t = cst.tile([128, 128], F16, tag="ident")
        nc.sync.dma_start(out=ident_t, in_=ident_in)
        ones_t = cst.tile([128, 128], F16, tag="ones")
        nc.sync.dma_start(out=ones_t, in_=ones_in)
        hmask_t = cst.tile([128, KE, H], F16, tag="hmask")
        nc.sync.dma_start(out=hmask_t,
                          in_=hmask_in.rearrange("(k p) h -> p k h", p=128))
        cmask_t = cst.tile([H, KE, 128], F16, tag="cmask")
        nc.sync.dma_start(out=cmask_t,
                          in_=cmask_in.rearrange("h (k p) -> h k p", p=128))
        ONES_COL = ones_t[:, 0:1]
        ONES_ROW = ones_t[0:1, :]
        eps_den = cst.tile([8, 1], F32, tag="epsd")
        v.memset(eps_den, 1e-6 * DEN_SCALE)
        eps_ln = cst.tile([1, 1], F32, tag="epsl")
        v.memset(eps_ln, LN_EPS)

        def col_tile(src_ap, m, tag="col"):
            t = wcol.tile([128, m], F32, tag=tag)
            nc.sync.dma_start(out=t, in_=src_ap.rearrange("(m p) -> p m", p=128))
            return t

        def ln_pair(jobs, relu=False):
            """LayerNorm over features, pair-interleaved.
            jobs: list of (xs, g_col, b_col, outt); xs/outt fp16
            [128, KE, C]; outt doubles as x^2 scratch."""
            for xs, _, _, outt in jobs:
                v.tensor_tensor(out=outt, in0=xs, in1=xs, op=ALU.mult)
            pst = []
            for xs, _, _, outt in jobs:
                pp_s = psst.tile([8, C], F32, tag="st", name="lnst")
                pp_q = psst.tile([8, C], F32, tag="st", name="lnsq")
                for m in range(KE):
                    mm(pp_s[0:1, :], ONES_COL, xs[:, m, :], start=(m == 0),
                       stop=(m == KE - 1))
                for m in range(KE):
                    mm(pp_q[0:1, :], ONES_COL, outt[:, m, :], start=(m == 0),
                       stop=(m == KE - 1))
                pst.append((pp_s, pp_q))
            stts = []
            for (xs, _, _, _), (pp_s, pp_q) in zip(jobs, pst):
                arow = pc.tile([1, C], F32, tag="s2", bufs=4, name="arow")
                brow = pc.tile([1, C], F32, tag="s2", bufs=4, name="brow")
                sc.activation(arow, pp_s[0:1, :], AF.Copy, scale=1.0 / E)
                sc.activation(brow, pp_q[0:1, :], AF.Copy, scale=1.0 / E)
                trow2 = pc.tile([1, C], F32, tag="s2", bufs=4)
                sc.activation(trow2, arow, AF.Square)
                v.tensor_tensor(out=brow, in0=brow, in1=trow2,
                                op=ALU.subtract)
                sc.activation(brow, brow, AF.Sqrt, bias=eps_ln[0:1, 0:1])
                stt = pc.tile([1, 2, C], F16, tag="s2", bufs=4)
                v.reciprocal(out=stt[:, 0, :], in_=brow)
                v.tensor_tensor(out=stt[:, 1, :], in0=arow, in1=stt[:, 0, :],
                                op=ALU.mult)
                stts.append(stt)
            sbs = []
            for stt in stts:
                bc_s = ps.tile([128, C], F32, tag="mm")
                mm(bc_s, ones_t[0:1, :], stt[0:1, 0, :], start=True, stop=True)
                bc_t = ps.tile([128, C], F32, tag="mm")
                mm(bc_t, ones_t[0:1, :], stt[0:1, 1, :], start=True, stop=True)
                sb = pc.tile([128, 2, C], F16, tag="c2", bufs=3)
                sc.activation(sb[:, 0, :], bc_s, AF.Copy)
                sc.activation(sb[:, 1, :], bc_t, AF.Copy)
                sbs.append(sb)
            fn = AF.Relu if relu else AF.Identity
            for m in range(KE):
                for (xs, g_col, b_col, outt), sb in zip(jobs, sbs):
                    t1 = pc.tile([128, C], F16, tag="c1")
                    v.tensor_tensor(out=t1, in0=xs[:, m, :], in1=sb[:, 0, :],
                                    op=ALU.mult)
                    v.tensor_tensor(out=t1, in0=t1, in1=sb[:, 1, :],
                                    op=ALU.subtract)
                    sc.activation(outt[:, m, :], t1, fn,
                                  bias=b_col[:, m:m + 1],
                                  scale=g_col[:, m:m + 1])

        def load_x_chunk(dram_l1, ci, tag="a4"):
            xt = pa.tile([128, KE, C], F16, tag=tag)
            nc.sync.dma_start(out=xt, in_=dram_l1[:, :, ci * C:(ci + 1) * C])
            return xt

        def store_chunk(dram_l1, ci, t):
            gp.dma_start(out=dram_l1[:, :, ci * C:(ci + 1) * C], in_=t)

        # ---- entry transpose (interleaved) ----
        def entry_tile(x_ap, dst, ttk):
            x2 = pa.tile([128, E], F16, tag="a4")
            nc.sync.dma_start(out=x2, in_=x_ap[ttk * 128:(ttk + 1) * 128, :])
            xt = pa.tile([128, KE, 128], F16, tag="a4")
            for f in range(KE):
                pt = ps.tile([128, 128], F16, tag="mm")
                nc.tensor.transpose(pt, x2[:, f * 128:(f + 1) * 128], ident_t)
                sc.activation(xt[:, f, :], pt, AF.Copy)
            nc.sync.dma_start(out=dst[:, :, ttk * 128:(ttk + 1) * 128], in_=xt)

        PHASES.append(("entry", len(nc.inst_map)))
        for ttk in range(N // 128):
            entry_tile(body_feats, rs["b", 0], ttk)
            entry_tile(limb_feats, rs["l", 0], ttk)

        # ---- linear attention (pairs) ----
        def attn_setup(l, a):
            st = {}
            qt = wq.tile([128, KE, 3, E], F16, tag="qkv")
            for t3 in range(3):
                nc.sync.dma_start(
                    out=qt[:, :, t3, :],
                    in_=qkvw[l, a, t3].rearrange("(k p) e -> p k e", p=128))
            owt = wq.tile([128, KE, E], F16, tag="ow")
            nc.sync.dma_start(
                out=owt, in_=ow[l, a].rearrange("(k p) e -> p k e", p=128))
            st["qt"], st["owt"] = qt, owt
            st["ubq_col"] = col_tile(ub[l, a, 0], KE)
            ubkv = wrow.tile([1, 2, E], F16, tag="row")
            nc.sync.dma_start(out=ubkv[:, 0, :], in_=ub16[l, a, 1][None, :])
            nc.sync.dma_start(out=ubkv[:, 1, :], in_=ub16[l, a, 2][None, :])
            st["ubkv"] = ubkv
            st["ob_col"] = col_tile(ob[l, a], KE)
            st["kv_acc"] = pat.tile([128, 4, 258], F32, tag="kva",
                                    name="kv_acc")
            return st

        def alpha_pair_step(sts, srcs, ci):
            xts = [load_x_chunk(src, ci) for src in srcs]
            work = []
            for st, xt in zip(sts, xts):
                k2f = pa.tile([128, NTT, E], F16, tag="a4", name="k2f")
                v2x = pa.tile([128, NTT, 2, 258], F16, tag="a4", name="v2x")
                v.memset(v2x[:, :, :, 256:258], 1.0)
                work.append((st, xt, k2f, v2x))
            for tt in range(NTT):
                for st, xt, k2f, v2x in work:
                    qt = st["qt"]
                    pk = ps.tile([128, E], F32, tag="mm")
                    pv = ps.tile([128, E], F32, tag="mm")
                    for k in range(KE):
                        lx = xt[:, k, tt * 128:(tt + 1) * 128]
                        mm(pk, lx, qt[:, k, 1, :], start=(k == 0), stop=False)
                        mm(pv, lx, qt[:, k, 2, :], start=(k == 0), stop=False)
                    mm(pk, ONES_ROW, st["ubkv"][:, 0, :], start=False,
                       stop=True)
                    mm(pv, ONES_ROW, st["ubkv"][:, 1, :], start=False,
                       stop=True)
                    ee = pc.tile([128, E], F16, tag="c1")
                    rr = pc.tile([128, E], F16, tag="c1")
                    sc.activation(ee, pk, AF.Exp)
                    v.tensor_scalar_max(rr, pk, 0.0)
                    v.tensor_scalar_min(ee, ee, 1.0)
                    v.tensor_tensor(out=k2f[:, tt, :], in0=ee, in1=rr,
                                    op=ALU.add)
                    sc.activation(v2x[:, tt, 0, 0:256], pv[:, 0:256], AF.Copy)
                    sc.activation(v2x[:, tt, 1, 0:256], pv[:, 256:512],
                                  AF.Copy)
            for p in range(4):
                for st, xt, k2f, v2x in work:
                    pkv = pskv.tile([128, 258], F32, tag="kv")
                    for tt in range(NTT):
                        mm(pkv, k2f[:, tt, p * 128:(p + 1) * 128],
                           v2x[:, tt, p // 2, :],
                           start=(tt == 0), stop=(tt == NTT - 1))
                    kv_acc = st["kv_acc"]
                    if ci == 0:
                        sc.activation(kv_acc[:, p, :], pkv, AF.Copy)
                    else:
                        v.tensor_tensor(out=kv_acc[:, p, :],
                                        in0=kv_acc[:, p, :], in1=pkv,
                                        op=ALU.add)

        def alpha_fin(st):
            kv_acc = st["kv_acc"]
            bd = pat.tile([128, KE, 128], F16, tag="bd")
            v.memset(bd, 0.0)
            for p in range(4):
                h0c = (2 * p % 4) * 64
                h1c = ((2 * p + 1) % 4) * 64
                v.tensor_scalar_mul(bd[0:64, p, 0:64],
                                    kv_acc[0:64, p, h0c:h0c + 64], DEN_SCALE)
                v.tensor_scalar_mul(bd[64:128, p, 64:128],
                                    kv_acc[64:128, p, h1c:h1c + 64], DEN_SCALE)
            kmm = pat.tile([128, KE, H], F16, tag="km")
            for k in range(KE):
                v.tensor_scalar_mul(kmm[:, k, :], hmask_t[:, k, :],
                                    kv_acc[:, k, 256:257])
            st["bd"], st["kmm"] = bd, kmm

        def beta_pair_step(sts, srcs, ci, tails):
            xqs = [load_x_chunk(src, ci) for src in srcs]
            qfs = [pa.tile([128, KE, C], F16, tag="a4", name="qf")
                   for _ in sts]
            for m in range(KE):
                for st, xq, qf in zip(sts, xqs, qfs):
                    qt = st["qt"]
                    pq = ps.tile([128, C], F32, tag="mm")
                    for k in range(KE):
                        mm(pq, qt[:, k, 0, m * 128:(m + 1) * 128],
                           xq[:, k, :], start=(k == 0), stop=(k == KE - 1))
                    ee = pc.tile([128, C], F16, tag="c1")
                    rr = pc.tile([128, C], F16, tag="c1")
                    sc.activation(ee, pq, AF.Exp,
                                  bias=st["ubq_col"][:, m:m + 1])
                    v.tensor_scalar(out=rr, in0=pq,
                                    scalar1=st["ubq_col"][:, m:m + 1],
                                    scalar2=0.0, op0=ALU.add, op1=ALU.max)
                    v.tensor_scalar_min(ee, ee, 1.0)
                    v.tensor_tensor(out=qf[:, m, :], in0=ee, in1=rr,
                                    op=ALU.add)
            recs = []
            for st, qf in zip(sts, qfs):
                pd = psst.tile([8, C], F32, tag="st", name="pd")
                for k in range(KE):
                    mm(pd, st["kmm"][:, k, :], qf[:, k, :], start=(k == 0),
                       stop=(k == KE - 1))
                den = pc.tile([8, C], F32, tag="s2", bufs=4)
                sc.activation(den, pd, AF.Identity, bias=eps_den,
                              scale=DEN_SCALE)
                rec = pc.tile([8, C], F16, tag="s2", bufs=4)
                v.reciprocal(out=rec, in_=den)
                recs.append(rec)
            atts = [pa.tile([128, KE, C], F16, tag="a4", name="att")
                    for _ in sts]
            for m in range(KE):
                for st, qf, att, rec in zip(sts, qfs, atts, recs):
                    pn = ps.tile([128, C], F32, tag="mm")
                    mm(pn, st["bd"][:, m, :], qf[:, m, :], start=True,
                       stop=True)
                    pr = ps.tile([128, C], F32, tag="mm")
                    mm(pr, cmask_t[:, m, :], rec, start=True, stop=True)
                    rb = pc.tile([128, C], F16, tag="c1")
                    sc.activation(rb, pr, AF.Copy)
                    v.tensor_tensor(out=att[:, m, :], in0=pn, in1=rb,
                                    op=ALU.mult)
            projs = [pa.tile([128, KE, C], F16, tag="a4", name="proj")
                     for _ in sts]
            for m in range(KE):
                for st, att, proj in zip(sts, atts, projs):
                    po = ps.tile([128, C], F32, tag="mm")
                    for k in range(KE):
                        mm(po, st["owt"][:, k, m * 128:(m + 1) * 128],
                           att[:, k, :], start=(k == 0), stop=(k == KE - 1))
                    sc.activation(proj[:, m, :], po, AF.Identity,
                                  bias=st["ob_col"][:, m:m + 1])
            tails[0](ci, projs, xqs)

        # ---- tails (pair) ----
        def make_self_tail_pair(l, dsts):
            cols = []
            for i, s in enumerate(("b", "l")):
                g_col = col_tile(lng[l, i], KE, tag="lncol")
                b_col = col_tile(lnb[l, i], KE, tag="lncol")
                cols.append((g_col, b_col))

            def tail(ci, projs, xqs):
                jobs = []
                for (g_col, b_col), proj, xq, dst in zip(cols, projs, xqs,
                                                         dsts):
                    v.tensor_tensor(out=proj, in0=proj, in1=xq, op=ALU.add)
                for (g_col, b_col), proj, xq, dst in zip(cols, projs, xqs,
                                                         dsts):
                    outt = pa.tile([128, KE, C], F16, tag="a4", name="outt")
                    jobs.append((proj, g_col, b_col, outt))
                ln_pair(jobs)
                for (j, dst) in zip(jobs, dsts):
                    store_chunk(dst, ci, j[3])

            return tail

        def make_cross_tail_pair(l, dsts):
            gw1t = wq.tile([128, 2 * KE, E4], F16, tag="gw")
            nc.sync.dma_start(out=gw1t,
                              in_=gw1[l].rearrange("(k p) g -> p k g", p=128))
            gwd_col = wcol.tile([128, 1], F16, tag="gwd")
            nc.sync.dma_start(out=gwd_col,
                              in_=gwd[l].rearrange("(m p) -> p m", p=128))
            gb1_col = col_tile(gb1[l], 1, tag="lncol")
            gb2d_t = wcol.tile([1, 1], F32, tag="gb2d")
            nc.sync.dma_start(out=gb2d_t, in_=gb2d[l][None, :])
            g_col = col_tile(lng[l, 2], KE, tag="lncol")
            b_col = col_tile(lnb[l, 2], KE, tag="lncol")

            def tail(ci, projs, xqs):
                bgts = []
                for proj, xq in zip(projs, xqs):
                    pg = ps.tile([128, C], F32, tag="mm")
                    for k in range(2 * KE):
                        rhs = xq[:, k, :] if k < KE else proj[:, k - KE, :]
                        mm(pg, gw1t[:, k, :], rhs, start=(k == 0),
                           stop=(k == 2 * KE - 1))
                    g1f = pc.tile([128, C], F16, tag="c1")
                    sc.activation(g1f, pg, AF.Relu, bias=gb1_col[:, 0:1])
                    g1t = pc.tile([128, C], F16, tag="c1")
                    v.tensor_scalar_min(g1t, g1f, 6.0)
                    pg2 = psst.tile([8, C], F32, tag="st", name="pg2")
                    mm(pg2[0:1, :], gwd_col, g1t, start=True, stop=True)
                    bgf = pc.tile([1, C], F16, tag="s2", bufs=4)
                    sc.activation(bgf, pg2[0:1, :], AF.Sigmoid,
                                  bias=gb2d_t[0:1, 0:1])
                    pbg = ps.tile([128, C], F32, tag="mm")
                    mm(pbg, ones_t[0:1, :], bgf, start=True, stop=True)
                    bgt = pc.tile([128, C], F16, tag="c1")
                    sc.activation(bgt, pbg, AF.Copy)
                    bgts.append(bgt)
                jobs = []
                for proj, xq, bgt in zip(projs, xqs, bgts):
                    mt = pa.tile([128, KE, C], F16, tag="a4", name="mt")
                    v.tensor_tensor(out=mt, in0=xq, in1=proj, op=ALU.subtract)
                    for m in range(KE):
                        v.tensor_tensor(out=mt[:, m, :], in0=mt[:, m, :],
                                        in1=bgt, op=ALU.mult)
                    v.tensor_tensor(out=mt, in0=mt, in1=proj, op=ALU.add)
                    outt = pa.tile([128, KE, C], F16, tag="a4", name="outt")
                    jobs.append((mt, g_col, b_col, outt))
                ln_pair(jobs)
                for (j, dst) in zip(jobs, dsts):
                    store_chunk(dst, ci, j[3])

            return tail

        # ---- FFN pair ----
        def ffn_setup(l, s):
            si = 0 if s == "b" else 1
            st = {}
            w1t = wq.tile([128, KE, X], F16, tag="w1")
            nc.sync.dma_start(
                out=w1t, in_=w1[l, si].rearrange("(k p) x -> p k x", p=128))
            w2t = wq.tile([128, KX, E], F16, tag="w2")
            nc.sync.dma_start(
                out=w2t, in_=w2[l, si].rearrange("(k p) e -> p k e", p=128))
            tapt = wq.tile([128, KX, 3, 128], F16, tag="tp" + s, bufs=1)
            nc.sync.dma_start(out=tapt,
                              in_=taps[l, si].rearrange("m t p f -> p m t f"))
            st["w1t"], st["w2t"], st["tapt"] = w1t, w2t, tapt
            st["b1_col"] = col_tile(b1[l, si], KX, tag="ffcol")
            st["b2_col"] = col_tile(b2[l, si], KE, tag="ffcol")
            st["B_col"] = col_tile(convB[l, si], KX, tag="ffcol")
            st["g_col"] = col_tile(lng[l, 3 if s == "b" else 4], KE,
                                   tag="lncol")
            st["bb_col"] = col_tile(lnb[l, 3 if s == "b" else 4], KE,
                                    tag="lncol")
            st["hts"] = [None] * NC
            st["xts"] = [None] * NC
            return st

        def ffn_h_pair(sts, srcs, ci):
            for st, src in zip(sts, srcs):
                xt = load_x_chunk(src, ci)
                st["xts"][ci] = xt
                ht = pb.tile([128, KX, C + 2], F16, tag="ht")
                if ci == 0:
                    v.memset(ht[:, :, 0:1], 0.0)
                st["hts"][ci] = ht
            for m in range(KX):
                for st in sts:
                    ht, xt = st["hts"][ci], st["xts"][ci]
                    ph = ps.tile([128, C], F32, tag="mm")
                    for k in range(KE):
                        mm(ph, st["w1t"][:, k, m * 128:(m + 1) * 128],
                           xt[:, k, :], start=(k == 0), stop=(k == KE - 1))
                    sc.activation(ht[:, m, 1:C + 1], ph, AF.Relu,
                                  bias=st["b1_col"][:, m:m + 1])
                    v.tensor_scalar_min(ht[:, m, 1:C + 1], ht[:, m, 1:C + 1],
                                        6.0)
            for st in sts:
                ht = st["hts"][ci]
                prev = st["hts"][ci - 1] if ci > 0 else None
                if prev is not None:
                    v.tensor_copy(prev[:, :, C + 1:C + 2], ht[:, :, 1:2])
                    v.tensor_copy(ht[:, :, 0:1], prev[:, :, C:C + 1])
                if ci == NC - 1:
                    v.memset(ht[:, :, C + 1:C + 2], 0.0)

        def ffn_tail_pair(sts, dsts, ci):
            h2s = []
            for st in sts:
                h2 = pb.tile([128, KX, C], F16, tag="h2", bufs=2, name="h2")
                h2s.append(h2)
            for m in range(KX):
                for st, h2 in zip(sts, h2s):
                    ht = st["hts"][ci]
                    pacc = ps.tile([128, C], F32, tag="mm")
                    for t in range(3):
                        mm(pacc, st["tapt"][:, m, t, :], ht[:, m, t:t + C],
                           start=(t == 0), stop=(t == 2))
                    rel = pc.tile([128, C], F16, tag="c1")
                    sc.activation(rel, pacc, AF.Relu,
                                  bias=st["B_col"][:, m:m + 1])
                    v.tensor_scalar_min(h2[:, m, :], rel, 6.0)
            rts = []
            for st, h2 in zip(sts, h2s):
                rt = pa.tile([128, KE, C], F16, tag="a4", name="rt")
                rts.append(rt)
            for m in range(KE):
                for st, h2, rt in zip(sts, h2s, rts):
                    pw = ps.tile([128, C], F32, tag="mm")
                    for k in range(KX):
                        mm(pw, st["w2t"][:, k, m * 128:(m + 1) * 128],
                           h2[:, k, :], start=(k == 0), stop=(k == KX - 1))
                    sc.activation(rt[:, m, :], pw, AF.Identity,
                                  bias=st["b2_col"][:, m:m + 1])
            jobs = []
            for st, rt, dst in zip(sts, rts, dsts):
                v.tensor_tensor(out=rt, in0=rt, in1=st["xts"][ci], op=ALU.add)
                outt = pa.tile([128, KE, C], F16, tag="a4", name="outt")
                jobs.append((rt, st["g_col"], st["bb_col"], outt))
            ln_pair(jobs)
            for j, dst in zip(jobs, dsts):
                store_chunk(dst, ci, j[3])
            for st in sts:
                st["hts"][ci] = st["xts"][ci] = None

        # ---- layers ----
        for l in range(L):
            bsrc = rs["b", 0] if l == 0 else rs["b", (l - 1, 3)]
            lsrc = rs["l", 0] if l == 0 else rs["l", (l - 1, 3)]

            PHASES.append((f"attnA{l}.alpha", len(nc.inst_map)))
            stA = [attn_setup(l, 0), attn_setup(l, 1)]
            for ci in range(NC):
                alpha_pair_step(stA, [bsrc, lsrc], ci)
            for st in stA:
                alpha_fin(st)
            PHASES.append((f"attnA{l}.beta", len(nc.inst_map)))
            tailA = make_self_tail_pair(l, [rs["b", (l, 1)], rs["l", (l, 1)]])
            for ci in range(NC):
                beta_pair_step(stA, [bsrc, lsrc], ci, [tailA])

            PHASES.append((f"attnB{l}.alpha", len(nc.inst_map)))
            b1d, l1d = rs["b", (l, 1)], rs["l", (l, 1)]
            stB = [attn_setup(l, 2), attn_setup(l, 3)]
            for ci in range(NC):
                alpha_pair_step(stB, [l1d, b1d], ci)
            for st in stB:
                alpha_fin(st)
            PHASES.append((f"attnB{l}.beta", len(nc.inst_map)))
            tailB = make_cross_tail_pair(l, [rs["b", (l, 2)],
                                             rs["l", (l, 2)]])
            for ci in range(NC):
                beta_pair_step(stB, [b1d, l1d], ci, [tailB])

            PHASES.append((f"ffn{l}", len(nc.inst_map)))
            stF = [ffn_setup(l, "b"), ffn_setup(l, "l")]
            fsrc = [rs["b", (l, 2)], rs["l", (l, 2)]]
            fdst = [rs["b", (l, 3)], rs["l", (l, 3)]]
            ffn_h_pair(stF, fsrc, 0)
            for ci in range(1, NC):
                ffn_h_pair(stF, fsrc, ci)
                ffn_tail_pair(stF, fdst, ci - 1)
            ffn_tail_pair(stF, fdst, NC - 1)

        PHASES.append(("final", len(nc.inst_map)))
        # ---- final head ----
        fw1t = wq.tile([128, 2 * KE, E2], F16, tag="w1")
        nc.sync.dma_start(out=fw1t,
                          in_=fw1.rearrange("(k p) g -> p k g", p=128))
        fw2t = wq.tile([128, 2, E], F16, tag="gw")
        nc.sync.dma_start(out=fw2t,
                          in_=fw2.rearrange("(k p) e -> p k e", p=128))
        rw1t = wq.tile([128, KE, E4], F16, tag="gw")
        nc.sync.dma_start(out=rw1t,
                          in_=rw1.rearrange("(k p) g -> p k g", p=128))
        rw2t = wrow.tile([128, E8], F16, tag="row2")
        nc.sync.dma_start(out=rw2t, in_=rw2)
        rw3t = wrow.tile([E8, 16], F16, tag="row2")
        nc.sync.dma_start(out=rw3t, in_=rw3p)
        rb3_row = wrow.tile([1, 16], F16, tag="row")
        nc.sync.dma_start(out=rb3_row, in_=rb3p)
        fb1_col = col_tile(fb1, 2, tag="fcol")
        fb2_col = col_tile(fb2, KE, tag="fcol")
        flng_col = col_tile(flng, KE, tag="fcol")
        flnb_col = col_tile(flnb, KE, tag="fcol")
        rb1_col = col_tile(rb1, 1, tag="fcol")
        rb2_col = wcol.tile([E8, 1], F32, tag="fcol")
        nc.sync.dma_start(out=rb2_col, in_=rb2[:, None])
        out_ap = out_dram.ap()

        bsrc, lsrc = rs["b", (L - 1, 3)], rs["l", (L - 1, 3)]
        for ci in range(NC):
            xb = load_x_chunk(bsrc, ci)
            xl = load_x_chunk(lsrc, ci)
            f1t = [pc.tile([128, C], F16, tag="c1", name=f"f1t{_i}")
                   for _i in range(2)]
            for m in range(2):
                pf = ps.tile([128, C], F32, tag="mm")
                for k in range(2 * KE):
                    rhs = xb[:, k, :] if k < KE else xl[:, k - KE, :]
                    mm(pf, fw1t[:, k, m * 128:(m + 1) * 128], rhs,
                       start=(k == 0), stop=(k == 2 * KE - 1))
                f1f = pc.tile([128, C], F16, tag="c1")
                sc.activation(f1f, pf, AF.Relu, bias=fb1_col[:, m:m + 1])
                v.tensor_scalar_min(f1t[m], f1f, 6.0)
            ft = pa.tile([128, KE, C], F16, tag="a4")
            for m in range(KE):
                pf2 = ps.tile([128, C], F32, tag="mm")
                for k in range(2):
                    mm(pf2, fw2t[:, k, m * 128:(m + 1) * 128],
                       f1t[k], start=(k == 0), stop=(k == 1))
                sc.activation(ft[:, m, :], pf2, AF.Identity,
                              bias=fb2_col[:, m:m + 1])
            frt = pa.tile([128, KE, C], F16, tag="a4")
            ln_pair([(ft, flng_col, flnb_col, frt)], relu=True)
            p1 = ps.tile([128, C], F32, tag="mm")
            for k in range(KE):
                mm(p1, rw1t[:, k, :], frt[:, k, :], start=(k == 0),
                   stop=(k == KE - 1))
            h1f = pc.tile([128, C], F16, tag="c1")
            sc.activation(h1f, p1, AF.Relu, bias=rb1_col[:, 0:1])
            h1t = pc.tile([128, C], F16, tag="c1")
            v.tensor_scalar_min(h1t, h1f, 6.0)
            p2 = ps.tile([E8, C], F32, tag="mm")
            mm(p2, rw2t, h1t, start=True, stop=True)
            h2f = pc.tile([E8, C], F16, tag="c1")
            sc.activation(h2f, p2, AF.Relu, bias=rb2_col[:, 0:1])
            h2t = pc.tile([E8, C], F16, tag="c1")
            v.tensor_scalar_min(h2t, h2f, 6.0)
            ot = pc.tile([128, NTT, c.OUT], F32, tag="c2", bufs=3)
            for tt in range(NTT):
                p3 = ps.tile([128, 16], F32, tag="mm")
                mm(p3, h2t[:, tt * 128:(tt + 1) * 128], rw3t,
                   start=True, stop=False)
                mm(p3, ONES_ROW[:, 0:128], rb3_row, start=False, stop=True)
                sc.activation(ot[:, tt, :], p3[:, 0:c.OUT], AF.Copy)
            nc.sync.dma_start(
                out=out_ap[ci * C:(ci + 1) * C, :].rearrange(
                    "(tt p) o -> p tt o", p=128),
                in_=ot)

    return din, out_dram


# ======================================================================
# kernel() entry point: full inputs in, full outputs out (8-core SPMD).
# ======================================================================
import concourse.bacc as _bacc
from concourse.bass_utils import run_bass_kernel_spmd as _run_spmd

_N_CORES = 8
_CACHE = {}


def _get_nc():
    if "nc" not in _CACHE:
        nc = _bacc.Bacc("TRN2", target_bir_lowering=False, debug=False)
        build(nc, Cfg())
        nc.finalize()
        _CACHE["nc"] = nc
    return _CACHE["nc"]


def kernel(**inputs):
    nc = _get_nc()
    cfg = Cfg()
    arr = {k: np.asarray(v) for k, v in inputs.items()}
    consts = host_constants(cfg, arr)
    shared = {k: a for k, a in consts.items()
              if k not in ("body_feats", "limb_feats")}
    in_maps = []
    for i in range(_N_CORES):
        m = dict(shared)
        m["body_feats"] = np.ascontiguousarray(consts["body_feats"][i])
        m["limb_feats"] = np.ascontiguousarray(consts["limb_feats"][i])
        in_maps.append(m)
    res = run_kernel_spmd_cached(nc, in_maps)
    out = np.stack([res[i]["out"] for i in range(_N_CORES)], axis=0)
    return out.astype(np.float32)


def run_kernel_spmd_cached(nc, in_maps, **kw):
    r = _run_spmd(nc, in_maps, list(range(_N_CORES)), **kw)
    _CACHE["last_result"] = r
    return r.results


# revision 29
# speedup vs baseline: 1.7823x; 1.0290x over previous
"""Dual-stream linear-attention transformer kernel (per-core), v3.

v3: per-m interleaved emission across the body/limb pair (ACT has no
exec-queue lookahead, so fine-grained alternation is what keeps it fed),
LN stats pairs share one PSUM tile (rows 0/1), fp16 operands throughout.
See v2 notes below.

  - fp16 matmul operands + activations + residual DRAM (fp32 PSUM/stats).
  - q/k/v low-rank projections premultiplied on host to single [E,E]
    mats; k/v produced directly token-major by using x as lhsT.
  - depthwise conv on PE via host-built diagonal tap matrices (BN scale
    folded in) against halo-padded h tiles.
  - attention denominator scaled by 1/64 so fp16 reciprocals stay in
    the normal range (1/64 folded into the bd kv blocks).

Layouts:
  - layout 1: [E, N] feature-major; SBUF tiles [128, KE, C].
  - layout 2 (k/v only): [tok, E] token-major.
  - Residuals in internal DRAM as fp16 [E, N] -> p k n.
"""

from dataclasses import dataclass
from contextlib import ExitStack

import numpy as np

import concourse.bass as bass
import concourse.mybir as mybir
import concourse.tile as tile

F32 = mybir.dt.float32
F16 = mybir.dt.float16
AF = mybir.ActivationFunctionType
ALU = mybir.AluOpType

LN_EPS = 1e-5
BN_EPS = 1e-5
DEN_SCALE = 1.0 / 64.0


@dataclass
class Cfg:
    N: int = 2048
    E: int = 512
    R: int = 256
    X: int = 1024
    H: int = 8
    L: int = 3
    OUT: int = 15
    C: int = 512

    @property
    def KE(self):
        return self.E // 128

    @property
    def KX(self):
        return self.X // 128

    @property
    def NC(self):
        return self.N // self.C

    @property
    def NTT(self):
        return self.C // 128


def host_constants(cfg, inputs):
    """Precompute fp16 weights / fused constants on host."""
    c = cfg
    f = lambda a: np.ascontiguousarray(a, dtype=np.float32)
    h = lambda a: np.ascontiguousarray(a, dtype=np.float16)
    dw, uw = f(inputs["dw"]), f(inputs["uw"])
    qkvw = np.einsum("latir,latrj->latij", dw, uw)
    out = {
        "body_feats": h(inputs["body_feats"]),
        "limb_feats": h(inputs["limb_feats"]),
        "qkvw": h(qkvw),
        "ub": f(inputs["ub"]),
        "ub16": h(inputs["ub"]),
        "ow": h(inputs["ow"]),
        "ob": f(inputs["ob"]),
        "w1": h(inputs["w1"]),
        "b1": f(inputs["b1"]),
        "w2": h(inputs["w2"]),
        "b2": f(inputs["b2"]),
        "lng": f(inputs["lng"]),
        "lnb": f(inputs["lnb"]),
        "gw1": h(inputs["gw1"]),
        "gb1": f(inputs["gb1"]),
        "gwd": h(f(inputs["gw2"])[:, :, 0] - f(inputs["gw2"])[:, :, 1]),
        "gb2d": f(f(inputs["gb2"])[:, 0:1] - f(inputs["gb2"])[:, 1:2]),
        "fw1": h(inputs["fw1"]),
        "fb1": f(inputs["fb1"]),
        "fw2": h(inputs["fw2"]),
        "fb2": f(inputs["fb2"]),
        "flng": f(inputs["flng"]),
        "flnb": f(inputs["flnb"]),
        "rw1": h(inputs["rw1"]),
        "rb1": f(inputs["rb1"]),
        "rw2": h(inputs["rw2"]),
        "rb2": f(inputs["rb2"]),
    }
    rw3 = np.zeros((c.E // 8, 16), np.float16)
    rw3[:, : c.OUT] = f(inputs["rw3"])
    out["rw3p"] = rw3
    rb3 = np.zeros((1, 16), np.float16)
    rb3[0, : c.OUT] = f(inputs["rb3"])
    out["rb3p"] = rb3
    rsq = 1.0 / np.sqrt(1.0 + BN_EPS)
    A = f(inputs["bng"]) * rsq
    cw, cb = f(inputs["cw"]), f(inputs["cb"])
    taps = np.zeros((c.L, 2, c.KX, 3, 128, 128), np.float16)
    idx = np.arange(128)
    for t in range(3):
        wA = (cw[:, :, :, t] * A).reshape(c.L, 2, c.KX, 128)
        taps[:, :, :, t, idx, idx] = wA.astype(np.float16)
    out["taps"] = taps
    out["convB"] = f(cb * A + f(inputs["bnb"]))
    out["ident"] = np.eye(128, dtype=np.float16)
    out["ones16"] = np.ones((128, 128), np.float16)
    E, H = c.E, c.H
    dh = E // H
    hmask = np.zeros((E, H), np.float16)
    for ff in range(E):
        hmask[ff, ff // dh] = 1.0
    out["hmask"] = hmask
    out["cmask"] = np.ascontiguousarray(hmask.T)
    return out


PHASES = []


def build(nc, cfg):
    c = cfg
    E, X, H, N, C, L = c.E, c.X, c.H, c.N, c.C, c.L
    KE, KX, NC, NTT = c.KE, c.KX, c.NC, c.NTT
    E4, E2, E8 = E // 4, E // 2, E // 8

    din = {}

    def inp(name, shape, dt):
        din[name] = nc.dram_tensor(name, list(shape), dt, kind="ExternalInput")
        return din[name].ap()

    body_feats = inp("body_feats", (N, E), F16)
    limb_feats = inp("limb_feats", (N, E), F16)
    qkvw = inp("qkvw", (L, 4, 3, E, E), F16)
    ub = inp("ub", (L, 4, 3, E), F32)
    ub16 = inp("ub16", (L, 4, 3, E), F16)
    ow = inp("ow", (L, 4, E, E), F16)
    ob = inp("ob", (L, 4, E), F32)
    w1 = inp("w1", (L, 2, E, X), F16)
    b1 = inp("b1", (L, 2, X), F32)
    w2 = inp("w2", (L, 2, X, E), F16)
    b2 = inp("b2", (L, 2, E), F32)
    taps = inp("taps", (L, 2, KX, 3, 128, 128), F16)
    convB = inp("convB", (L, 2, X), F32)
    lng = inp("lng", (L, 5, E), F32)
    lnb = inp("lnb", (L, 5, E), F32)
    gw1 = inp("gw1", (L, 2 * E, E4), F16)
    gb1 = inp("gb1", (L, E4), F32)
    gwd = inp("gwd", (L, E4), F16)
    gb2d = inp("gb2d", (L, 1), F32)
    fw1 = inp("fw1", (2 * E, E2), F16)
    fb1 = inp("fb1", (E2,), F32)
    fw2 = inp("fw2", (E2, E), F16)
    fb2 = inp("fb2", (E,), F32)
    flng = inp("flng", (E,), F32)
    flnb = inp("flnb", (E,), F32)
    rw1 = inp("rw1", (E, E4), F16)
    rb1 = inp("rb1", (E4,), F32)
    rw2 = inp("rw2", (E4, E8), F16)
    rb2 = inp("rb2", (E8,), F32)
    rw3p = inp("rw3p", (E8, 16), F16)
    rb3p = inp("rb3p", (1, 16), F16)
    ident_in = inp("ident", (128, 128), F16)
    ones_in = inp("ones16", (128, 128), F16)
    hmask_in = inp("hmask", (E, H), F16)
    cmask_in = inp("cmask", (H, E), F16)

    out_dram = nc.dram_tensor("out", [N, c.OUT], F32, kind="ExternalOutput")

    def idram(name):
        return [nc.dram_tensor(f"{name}_c{ci}", [E, C], F16).ap().rearrange(
            "(k p) n -> p k n", p=128) for ci in range(NC)]

    rs = {}
    for s in ("b", "l"):
        rs[s, 0] = idram(f"r{s}0")
        for l in range(L):
            for st in (1, 2, 3):
                rs[s, (l, st)] = idram(f"r{s}_{l}_{st}")

    lowp = nc.allow_low_precision("fp16 activations within rel-err budget")

    with tile.TileContext(nc) as tc, ExitStack() as ctx, lowp:
        p_ = ctx.enter_context
        cst = p_(tc.tile_pool(name="cst", bufs=1))
        wq = p_(tc.tile_pool(name="wq", bufs=2))       # big weights
        wcol = p_(tc.tile_pool(name="wcol", bufs=8))   # bias cols
        wrow = p_(tc.tile_pool(name="wrow", bufs=2))   # bias rows
        pa = p_(tc.tile_pool(name="pa", bufs=10))      # 4KB fp16 act tiles
        pb = p_(tc.tile_pool(name="pb", bufs=4))       # ffn h tiles
        pc = p_(tc.tile_pool(name="pc", bufs=9))       # small tiles
        pat = p_(tc.tile_pool(name="pat", bufs=2))     # attn persistents
        ps = p_(tc.tile_pool(name="ps", bufs=5, space="PSUM"))
        pskv = p_(tc.tile_pool(name="pskv", bufs=1, space="PSUM"))
        psst = p_(tc.tile_pool(name="psst", bufs=2, space="PSUM"))

        v, sc, gp = nc.vector, nc.scalar, nc.gpsimd

        def mm(out, lhsT, rhs, start, stop):
            nc.tensor.matmul(out, lhsT, rhs, start=start, stop=stop)

        # ---- constants ----
        ident_t = cst.tile([128, 128], F16, tag="ident")
        nc.sync.dma_start(out=ident_t, in_=ident_in)
        ones_t = cst.tile([128, 128], F16, tag="ones")
        nc.sync.dma_start(out=ones_t, in_=ones_in)
        hmask_t = cst.tile([128, KE, H], F16, tag="hmask")
        nc.sync.dma_start(out=hmask_t,
                          in_=hmask_in.rearrange("(k p) h -> p k h", p=128))
        cmask_t = cst.tile([H, KE, 128], F16, tag="cmask")
        nc.sync.dma_start(out=cmask_t,
                          in_=cmask_in.rearrange("h (k p) -> h k p", p=128))
        ONES_COL = ones_t[:, 0:1]
        ONES_ROW = ones_t[0:1, :]
        eps_den = cst.tile([8, 1], F32, tag="epsd")
        v.memset(eps_den, 1e-6 * DEN_SCALE)
        eps_ln = cst.tile([1, 1], F32, tag="epsl")
        v.memset(eps_ln, LN_EPS)

        def col_tile(src_ap, m, tag="col"):
            t = wcol.tile([128, m], F32, tag=tag)
            nc.sync.dma_start(out=t, in_=src_ap.rearrange("(m p) -> p m", p=128))
            return t

        def ln_pair(jobs, relu=False, apply_dve=False):
            """LayerNorm over features, pair-interleaved.
            jobs: list of (xs, g_col, b_col, outt); xs/outt fp16
            [128, KE, C]; outt doubles as x^2 scratch."""
            for xs, _, _, outt in jobs:
                v.tensor_tensor(out=outt, in0=xs, in1=xs, op=ALU.mult)
            pst = []
            for xs, _, _, outt in jobs:
                pp_s = psst.tile([8, C], F32, tag="st", name="lnst")
                pp_q = psst.tile([8, C], F32, tag="st", name="lnsq")
                for m in range(KE):
                    mm(pp_s[0:1, :], ONES_COL, xs[:, m, :], start=(m == 0),
                       stop=(m == KE - 1))
                for m in range(KE):
                    mm(pp_q[0:1, :], ONES_COL, outt[:, m, :], start=(m == 0),
                       stop=(m == KE - 1))
                pst.append((pp_s, pp_q))
            stts = []
            for (xs, _, _, _), (pp_s, pp_q) in zip(jobs, pst):
                arow = pc.tile([1, C], F32, tag="s2", bufs=4, name="arow")
                brow = pc.tile([1, C], F32, tag="s2", bufs=4, name="brow")
                sc.activation(arow, pp_s[0:1, :], AF.Copy, scale=1.0 / E)
                sc.activation(brow, pp_q[0:1, :], AF.Copy, scale=1.0 / E)
                trow2 = pc.tile([1, C], F32, tag="s2", bufs=4)
                sc.activation(trow2, arow, AF.Square)
                v.tensor_tensor(out=brow, in0=brow, in1=trow2,
                                op=ALU.subtract)
                sc.activation(brow, brow, AF.Sqrt, bias=eps_ln[0:1, 0:1])
                stt = pc.tile([1, 2, C], F16, tag="s2", bufs=4)
                v.reciprocal(out=stt[:, 0, :], in_=brow)
                v.tensor_tensor(out=stt[:, 1, :], in0=arow, in1=stt[:, 0, :],
                                op=ALU.mult)
                stts.append(stt)
            sbs = []
            for stt in stts:
                bc_s = ps.tile([128, C], F32, tag="mm")
                mm(bc_s, ones_t[0:1, :], stt[0:1, 0, :], start=True, stop=True)
                bc_t = ps.tile([128, C], F32, tag="mm")
                mm(bc_t, ones_t[0:1, :], stt[0:1, 1, :], start=True, stop=True)
                sb = pc.tile([128, 2, C], F16, tag="c2", bufs=2)
                sc.activation(sb[:, 0, :], bc_s, AF.Copy)
                sc.activation(sb[:, 1, :], bc_t, AF.Copy)
                sbs.append(sb)
            fn = AF.Relu if relu else AF.Identity
            for m in range(KE):
                for (xs, g_col, b_col, outt), sb in zip(jobs, sbs):
                    t1 = pc.tile([128, C], F16, tag="c1")
                    v.tensor_tensor(out=t1, in0=xs[:, m, :], in1=sb[:, 0, :],
                                    op=ALU.mult)
                    v.tensor_tensor(out=t1, in0=t1, in1=sb[:, 1, :],
                                    op=ALU.subtract)
                    if apply_dve:
                        v.tensor_scalar(out=t1, in0=t1,
                                        scalar1=g_col[:, m:m + 1],
                                        scalar2=b_col[:, m:m + 1],
                                        op0=ALU.mult, op1=ALU.add)
                        if relu:
                            v.tensor_scalar_max(outt[:, m, :], t1, 0.0)
                        else:
                            v.tensor_copy(outt[:, m, :], t1)
                    else:
                        sc.activation(outt[:, m, :], t1, fn,
                                      bias=b_col[:, m:m + 1],
                                      scale=g_col[:, m:m + 1])

        def load_x_chunk(dram_l1, ci, tag="a4"):
            xt = pa.tile([128, KE, C], F16, tag=tag)
            nc.sync.dma_start(out=xt, in_=dram_l1[ci])
            return xt

        def store_chunk(dram_l1, ci, t):
            gp.dma_start(out=dram_l1[ci], in_=t)

        # ---- entry transpose (interleaved) ----
        def entry_tile(x_ap, dst, ttk):
            x2 = pa.tile([128, E], F16, tag="a4")
            nc.sync.dma_start(out=x2, in_=x_ap[ttk * 128:(ttk + 1) * 128, :])
            xt = pa.tile([128, KE, 128], F16, tag="a4")
            for f in range(KE):
                pt = ps.tile([128, 128], F16, tag="mm")
                nc.tensor.transpose(pt, x2[:, f * 128:(f + 1) * 128], ident_t)
                if f % 2 == 0:
                    sc.activation(xt[:, f, :], pt, AF.Copy)
                else:
                    v.tensor_copy(xt[:, f, :], pt)
            tl = ttk % NTT
            nc.sync.dma_start(out=dst[ttk // NTT][:, :, tl * 128:(tl + 1) * 128],
                              in_=xt)

        PHASES.append(("entry", len(nc.inst_map)))
        for ttk in range(N // 128):
            entry_tile(body_feats, rs["b", 0], ttk)
            entry_tile(limb_feats, rs["l", 0], ttk)

        # ---- linear attention (pairs) ----
        def attn_setup(l, a):
            st = {}
            kvw = wq.tile([128, KE, 2, E], F16, tag="kvw")
            for t3 in (1, 2):
                nc.sync.dma_start(
                    out=kvw[:, :, t3 - 1, :],
                    in_=qkvw[l, a, t3].rearrange("(k p) e -> p k e", p=128))
            qw = wq.tile([128, KE, E], F16, tag="qw")
            nc.sync.dma_start(
                out=qw, in_=qkvw[l, a, 0].rearrange("(k p) e -> p k e", p=128))
            owt = wq.tile([128, KE, E], F16, tag="ow")
            nc.sync.dma_start(
                out=owt, in_=ow[l, a].rearrange("(k p) e -> p k e", p=128))
            st["kvw"], st["qw"], st["owt"] = kvw, qw, owt
            st["ubq_col"] = col_tile(ub[l, a, 0], KE)
            ubkv = wrow.tile([1, 2, E], F16, tag="row")
            nc.sync.dma_start(out=ubkv[:, 0, :], in_=ub16[l, a, 1][None, :])
            nc.sync.dma_start(out=ubkv[:, 1, :], in_=ub16[l, a, 2][None, :])
            st["ubkv"] = ubkv
            st["ob_col"] = col_tile(ob[l, a], KE)
            st["kv_acc"] = pat.tile([128, 4, 258], F32, tag="kva",
                                    name="kv_acc")
            return st

        def alpha_pair_step(sts, srcs, ci):
            xts = [load_x_chunk(src, ci) for src in srcs]
            work = []
            for st, xt in zip(sts, xts):
                k2f = pa.tile([128, NTT, E], F16, tag="a4", name="k2f")
                v2x = pa.tile([128, NTT, 2, 258], F16, tag="a4", name="v2x")
                v.memset(v2x[:, :, :, 256:258], 1.0)
                work.append((st, xt, k2f, v2x))
            for tt in range(NTT):
                for st, xt, k2f, v2x in work:
                    kvw = st["kvw"]
                    pk = ps.tile([128, E], F32, tag="mm")
                    pv = ps.tile([128, E], F32, tag="mm")
                    for k in range(KE):
                        lx = xt[:, k, tt * 128:(tt + 1) * 128]
                        mm(pk, lx, kvw[:, k, 0, :], start=(k == 0), stop=False)
                        mm(pv, lx, kvw[:, k, 1, :], start=(k == 0), stop=False)
                    mm(pk, ONES_ROW, st["ubkv"][:, 0, :], start=False,
                       stop=True)
                    mm(pv, ONES_ROW, st["ubkv"][:, 1, :], start=False,
                       stop=True)
                    ee = pc.tile([128, E], F16, tag="c1")
                    rr = pc.tile([128, E], F16, tag="c1")
                    sc.activation(ee, pk, AF.Exp)
                    v.tensor_scalar_max(rr, pk, 0.0)
                    v.tensor_scalar_min(ee, ee, 1.0)
                    v.tensor_tensor(out=k2f[:, tt, :], in0=ee, in1=rr,
                                    op=ALU.add)
                    sc.activation(v2x[:, tt, 0, 0:256], pv[:, 0:256], AF.Copy)
                    sc.activation(v2x[:, tt, 1, 0:256], pv[:, 256:512],
                                  AF.Copy)
            for p in range(4):
                for st, xt, k2f, v2x in work:
                    pkv = pskv.tile([128, 258], F32, tag="kv")
                    for tt in range(NTT):
                        mm(pkv, k2f[:, tt, p * 128:(p + 1) * 128],
                           v2x[:, tt, p // 2, :],
                           start=(tt == 0), stop=(tt == NTT - 1))
                    kv_acc = st["kv_acc"]
                    if ci == 0:
                        sc.activation(kv_acc[:, p, :], pkv, AF.Copy)
                    else:
                        v.tensor_tensor(out=kv_acc[:, p, :],
                                        in0=kv_acc[:, p, :], in1=pkv,
                                        op=ALU.add)

        def alpha_fin(st):
            kv_acc = st["kv_acc"]
            bd = pat.tile([128, KE, 128], F16, tag="bd")
            v.memset(bd, 0.0)
            for p in range(4):
                h0c = (2 * p % 4) * 64
                h1c = ((2 * p + 1) % 4) * 64
                v.tensor_scalar_mul(bd[0:64, p, 0:64],
                                    kv_acc[0:64, p, h0c:h0c + 64], DEN_SCALE)
                v.tensor_scalar_mul(bd[64:128, p, 64:128],
                                    kv_acc[64:128, p, h1c:h1c + 64], DEN_SCALE)
            kmm = pat.tile([128, KE, H], F16, tag="km")
            for k in range(KE):
                v.tensor_scalar(out=kmm[:, k, :], in0=hmask_t[:, k, :],
                                scalar1=kv_acc[:, k, 256:257],
                                scalar2=DEN_SCALE, op0=ALU.mult, op1=ALU.mult)
            st["bd"], st["kmm"] = bd, kmm

        def beta_pair_step(sts, srcs, ci, tails):
            xqs = [load_x_chunk(src, ci) for src in srcs]
            qfs = [pa.tile([128, KE, C], F16, tag="a4", name="qf")
                   for _ in sts]
            pds = [psst.tile([8, C], F32, tag="st", name="pd")
                   for _ in sts]
            for m in range(KE):
                for st, xq, qf, pd in zip(sts, xqs, qfs, pds):
                    qw = st["qw"]
                    pq = ps.tile([128, C], F32, tag="mm")
                    for k in range(KE):
                        mm(pq, qw[:, k, m * 128:(m + 1) * 128],
                           xq[:, k, :], start=(k == 0), stop=(k == KE - 1))
                    ee = pc.tile([128, C], F16, tag="c1")
                    rr = pc.tile([128, C], F16, tag="c1")
                    sc.activation(ee, pq, AF.Exp,
                                  bias=st["ubq_col"][:, m:m + 1])
                    v.tensor_scalar(out=rr, in0=pq,
                                    scalar1=st["ubq_col"][:, m:m + 1],
                                    scalar2=0.0, op0=ALU.add, op1=ALU.max)
                    v.tensor_scalar_min(ee, ee, 1.0)
                    v.tensor_tensor(out=qf[:, m, :], in0=ee, in1=rr,
                                    op=ALU.add)
                    mm(pd, st["kmm"][:, m, :], qf[:, m, :], start=(m == 0),
                       stop=(m == KE - 1))
            recs = []
            for st, pd in zip(sts, pds):
                rec = pc.tile([8, C], F16, tag="s2", bufs=4)
                v.reciprocal(out=rec, in_=pd)
                recs.append(rec)
            atts = [pa.tile([128, KE, C], F16, tag="a4", name="att")
                    for _ in sts]
            for m in range(KE):
                for st, qf, att, rec in zip(sts, qfs, atts, recs):
                    pn = ps.tile([128, C], F32, tag="mm")
                    mm(pn, st["bd"][:, m, :], qf[:, m, :], start=True,
                       stop=True)
                    pr = ps.tile([128, C], F32, tag="mm")
                    mm(pr, cmask_t[:, m, :], rec, start=True, stop=True)
                    rb = pc.tile([128, C], F16, tag="c1")
                    sc.activation(rb, pr, AF.Copy)
                    v.tensor_tensor(out=att[:, m, :], in0=pn, in1=rb,
                                    op=ALU.mult)
            projs = [pa.tile([128, KE, C], F16, tag="a4", name="proj")
                     for _ in sts]
            for m in range(KE):
                for st, att, proj in zip(sts, atts, projs):
                    po = ps.tile([128, C], F32, tag="mm")
                    for k in range(KE):
                        mm(po, st["owt"][:, k, m * 128:(m + 1) * 128],
                           att[:, k, :], start=(k == 0), stop=(k == KE - 1))
                    sc.activation(proj[:, m, :], po, AF.Identity,
                                  bias=st["ob_col"][:, m:m + 1])
            tails[0](ci, projs, xqs)

        # ---- tails (pair) ----
        def make_self_tail_pair(l, dsts):
            cols = []
            for i, s in enumerate(("b", "l")):
                g_col = col_tile(lng[l, i], KE, tag="lncol")
                b_col = col_tile(lnb[l, i], KE, tag="lncol")
                cols.append((g_col, b_col))

            def tail(ci, projs, xqs):
                jobs = []
                for (g_col, b_col), proj, xq, dst in zip(cols, projs, xqs,
                                                         dsts):
                    v.tensor_tensor(out=proj, in0=proj, in1=xq, op=ALU.add)
                for (g_col, b_col), proj, xq, dst in zip(cols, projs, xqs,
                                                         dsts):
                    outt = pa.tile([128, KE, C], F16, tag="a4", name="outt")
                    jobs.append((proj, g_col, b_col, outt))
                ln_pair(jobs)
                for (j, dst) in zip(jobs, dsts):
                    store_chunk(dst, ci, j[3])

            return tail

        def make_cross_tail_pair(l, dsts):
            gw1t = wq.tile([128, 2 * KE, E4], F16, tag="gw")
            nc.sync.dma_start(out=gw1t,
                              in_=gw1[l].rearrange("(k p) g -> p k g", p=128))
            gwd_col = wcol.tile([128, 1], F16, tag="gwd")
            nc.sync.dma_start(out=gwd_col,
                              in_=gwd[l].rearrange("(m p) -> p m", p=128))
            gb1_col = col_tile(gb1[l], 1, tag="lncol")
            gb2d_t = wcol.tile([1, 1], F32, tag="gb2d")
            nc.sync.dma_start(out=gb2d_t, in_=gb2d[l][None, :])
            g_col = col_tile(lng[l, 2], KE, tag="lncol")
            b_col = col_tile(lnb[l, 2], KE, tag="lncol")

            def tail(ci, projs, xqs):
                bgts = []
                for proj, xq in zip(projs, xqs):
                    pg = ps.tile([128, C], F32, tag="mm")
                    for k in range(2 * KE):
                        rhs = xq[:, k, :] if k < KE else proj[:, k - KE, :]
                        mm(pg, gw1t[:, k, :], rhs, start=(k == 0),
                           stop=(k == 2 * KE - 1))
                    g1f = pc.tile([128, C], F16, tag="c1")
                    sc.activation(g1f, pg, AF.Relu, bias=gb1_col[:, 0:1])
                    g1t = pc.tile([128, C], F16, tag="c1")
                    v.tensor_scalar_min(g1t, g1f, 6.0)
                    pg2 = psst.tile([8, C], F32, tag="st", name="pg2")
                    mm(pg2[0:1, :], gwd_col, g1t, start=True, stop=True)
                    bgf = pc.tile([1, C], F16, tag="s2", bufs=4)
                    sc.activation(bgf, pg2[0:1, :], AF.Sigmoid,
                                  bias=gb2d_t[0:1, 0:1])
                    pbg = ps.tile([128, C], F32, tag="mm")
                    mm(pbg, ones_t[0:1, :], bgf, start=True, stop=True)
                    bgt = pc.tile([128, C], F16, tag="c1")
                    sc.activation(bgt, pbg, AF.Copy)
                    bgts.append(bgt)
                jobs = []
                for proj, xq, bgt in zip(projs, xqs, bgts):
                    mt = pa.tile([128, KE, C], F16, tag="a4", name="mt")
                    v.tensor_tensor(out=mt, in0=xq, in1=proj, op=ALU.subtract)
                    for m in range(KE):
                        v.tensor_tensor(out=mt[:, m, :], in0=mt[:, m, :],
                                        in1=bgt, op=ALU.mult)
                    v.tensor_tensor(out=mt, in0=mt, in1=proj, op=ALU.add)
                    outt = pa.tile([128, KE, C], F16, tag="a4", name="outt")
                    jobs.append((mt, g_col, b_col, outt))
                ln_pair(jobs)
                for (j, dst) in zip(jobs, dsts):
                    store_chunk(dst, ci, j[3])

            return tail

        # ---- FFN pair ----
        def ffn_setup(l, s):
            si = 0 if s == "b" else 1
            st = {}
            w1t = wq.tile([128, KE, X], F16, tag="w1")
            nc.sync.dma_start(
                out=w1t, in_=w1[l, si].rearrange("(k p) x -> p k x", p=128))
            w2t = wq.tile([128, KX, E], F16, tag="w2")
            nc.sync.dma_start(
                out=w2t, in_=w2[l, si].rearrange("(k p) e -> p k e", p=128))
            tapt = wq.tile([128, KX, 3, 128], F16, tag="tp" + s, bufs=1)
            nc.sync.dma_start(out=tapt,
                              in_=taps[l, si].rearrange("m t p f -> p m t f"))
            st["w1t"], st["w2t"], st["tapt"] = w1t, w2t, tapt
            st["b1_col"] = col_tile(b1[l, si], KX, tag="ffcol")
            st["b2_col"] = col_tile(b2[l, si], KE, tag="ffcol")
            st["B_col"] = col_tile(convB[l, si], KX, tag="ffcol")
            st["g_col"] = col_tile(lng[l, 3 if s == "b" else 4], KE,
                                   tag="lncol")
            st["bb_col"] = col_tile(lnb[l, 3 if s == "b" else 4], KE,
                                    tag="lncol")
            st["hts"] = [None] * NC
            st["xts"] = [None] * NC
            return st

        def ffn_h_pair(sts, srcs, ci):
            for st, src in zip(sts, srcs):
                xt = load_x_chunk(src, ci)
                st["xts"][ci] = xt
                ht = pb.tile([128, KX, C + 2], F16, tag="ht")
                if ci == 0:
                    v.memset(ht[:, :, 0:1], 0.0)
                st["hts"][ci] = ht
            for m in range(KX):
                for st in sts:
                    ht, xt = st["hts"][ci], st["xts"][ci]
                    ph = ps.tile([128, C], F32, tag="mm")
                    for k in range(KE):
                        mm(ph, st["w1t"][:, k, m * 128:(m + 1) * 128],
                           xt[:, k, :], start=(k == 0), stop=(k == KE - 1))
                    sc.activation(ht[:, m, 1:C + 1], ph, AF.Relu,
                                  bias=st["b1_col"][:, m:m + 1])
                    v.tensor_scalar_min(ht[:, m, 1:C + 1], ht[:, m, 1:C + 1],
                                        6.0)
            for st in sts:
                ht = st["hts"][ci]
                prev = st["hts"][ci - 1] if ci > 0 else None
                if prev is not None:
                    v.tensor_copy(prev[:, :, C + 1:C + 2], ht[:, :, 1:2])
                    v.tensor_copy(ht[:, :, 0:1], prev[:, :, C:C + 1])
                if ci == NC - 1:
                    v.memset(ht[:, :, C + 1:C + 2], 0.0)

        def ffn_tail_pair(sts, dsts, ci):
            h2s = []
            for st in sts:
                h2 = pb.tile([128, KX, C], F16, tag="h2", bufs=2, name="h2")
                h2s.append(h2)
            for m in range(KX):
                for st, h2 in zip(sts, h2s):
                    ht = st["hts"][ci]
                    pacc = ps.tile([128, C], F32, tag="mm")
                    for t in range(3):
                        mm(pacc, st["tapt"][:, m, t, :], ht[:, m, t:t + C],
                           start=(t == 0), stop=(t == 2))
                    rel = pc.tile([128, C], F16, tag="c1")
                    sc.activation(rel, pacc, AF.Relu,
                                  bias=st["B_col"][:, m:m + 1])
                    v.tensor_scalar_min(h2[:, m, :], rel, 6.0)
            rts = []
            for st, h2 in zip(sts, h2s):
                rt = pa.tile([128, KE, C], F16, tag="a4", name="rt")
                rts.append(rt)
            for m in range(KE):
                for st, h2, rt in zip(sts, h2s, rts):
                    pw = ps.tile([128, C], F32, tag="mm")
                    for k in range(KX):
                        mm(pw, st["w2t"][:, k, m * 128:(m + 1) * 128],
                           h2[:, k, :], start=(k == 0), stop=(k == KX - 1))
                    sc.activation(rt[:, m, :], pw, AF.Identity,
                                  bias=st["b2_col"][:, m:m + 1])
            jobs = []
            for st, rt, dst in zip(sts, rts, dsts):
                v.tensor_tensor(out=rt, in0=rt, in1=st["xts"][ci], op=ALU.add)
                outt = pa.tile([128, KE, C], F16, tag="a4", name="outt")
                jobs.append((rt, st["g_col"], st["bb_col"], outt))
            ln_pair(jobs)
            for j, dst in zip(jobs, dsts):
                store_chunk(dst, ci, j[3])
            for st in sts:
                st["hts"][ci] = st["xts"][ci] = None

        # ---- layers ----
        for l in range(L):
            bsrc = rs["b", 0] if l == 0 else rs["b", (l - 1, 3)]
            lsrc = rs["l", 0] if l == 0 else rs["l", (l - 1, 3)]

            PHASES.append((f"attnA{l}.alpha", len(nc.inst_map)))
            stA = [attn_setup(l, 0), attn_setup(l, 1)]
            for ci in range(NC):
                alpha_pair_step(stA, [bsrc, lsrc], ci)
            for st in stA:
                alpha_fin(st)
            PHASES.append((f"attnA{l}.beta", len(nc.inst_map)))
            tailA = make_self_tail_pair(l, [rs["b", (l, 1)], rs["l", (l, 1)]])
            for ci in range(NC):
                beta_pair_step(stA, [bsrc, lsrc], ci, [tailA])

            PHASES.append((f"attnB{l}.alpha", len(nc.inst_map)))
            b1d, l1d = rs["b", (l, 1)], rs["l", (l, 1)]
            stB = [attn_setup(l, 2), attn_setup(l, 3)]
            for ci in range(NC):
                alpha_pair_step(stB, [l1d, b1d], ci)
            for st in stB:
                alpha_fin(st)
            PHASES.append((f"attnB{l}.beta", len(nc.inst_map)))
            tailB = make_cross_tail_pair(l, [rs["b", (l, 2)],
                                             rs["l", (l, 2)]])
            for ci in range(NC):
                beta_pair_step(stB, [b1d, l1d], ci, [tailB])

            PHASES.append((f"ffn{l}", len(nc.inst_map)))
            stF = [ffn_setup(l, "b"), ffn_setup(l, "l")]
            fsrc = [rs["b", (l, 2)], rs["l", (l, 2)]]
            fdst = [rs["b", (l, 3)], rs["l", (l, 3)]]
            ffn_h_pair(stF, fsrc, 0)
            for ci in range(1, NC):
                ffn_h_pair(stF, fsrc, ci)
                ffn_tail_pair(stF, fdst, ci - 1)
            ffn_tail_pair(stF, fdst, NC - 1)

        PHASES.append(("final", len(nc.inst_map)))
        # ---- final head ----
        fw1t = wq.tile([128, 2 * KE, E2], F16, tag="w1")
        nc.sync.dma_start(out=fw1t,
                          in_=fw1.rearrange("(k p) g -> p k g", p=128))
        fw2t = wq.tile([128, 2, E], F16, tag="gw")
        nc.sync.dma_start(out=fw2t,
                          in_=fw2.rearrange("(k p) e -> p k e", p=128))
        rw1t = wq.tile([128, KE, E4], F16, tag="gw")
        nc.sync.dma_start(out=rw1t,
                          in_=rw1.rearrange("(k p) g -> p k g", p=128))
        rw2t = wrow.tile([128, E8], F16, tag="row2")
        nc.sync.dma_start(out=rw2t, in_=rw2)
        rw3t = wrow.tile([E8, 16], F16, tag="row2")
        nc.sync.dma_start(out=rw3t, in_=rw3p)
        rb3_row = wrow.tile([1, 16], F16, tag="row")
        nc.sync.dma_start(out=rb3_row, in_=rb3p)
        fb1_col = col_tile(fb1, 2, tag="fcol")
        fb2_col = col_tile(fb2, KE, tag="fcol")
        flng_col = col_tile(flng, KE, tag="fcol")
        flnb_col = col_tile(flnb, KE, tag="fcol")
        rb1_col = col_tile(rb1, 1, tag="fcol")
        rb2_col = wcol.tile([E8, 1], F32, tag="fcol")
        nc.sync.dma_start(out=rb2_col, in_=rb2[:, None])
        out_ap = out_dram.ap()

        bsrc, lsrc = rs["b", (L - 1, 3)], rs["l", (L - 1, 3)]
        for ci in range(NC):
            xb = load_x_chunk(bsrc, ci)
            xl = load_x_chunk(lsrc, ci)
            f1t = [pc.tile([128, C], F16, tag="c1", name=f"f1t{_i}")
                   for _i in range(2)]
            for m in range(2):
                pf = ps.tile([128, C], F32, tag="mm")
                for k in range(2 * KE):
                    rhs = xb[:, k, :] if k < KE else xl[:, k - KE, :]
                    mm(pf, fw1t[:, k, m * 128:(m + 1) * 128], rhs,
                       start=(k == 0), stop=(k == 2 * KE - 1))
                f1f = pc.tile([128, C], F16, tag="c1")
                v.tensor_scalar(out=f1f, in0=pf, scalar1=fb1_col[:, m:m + 1],
                                scalar2=0.0, op0=ALU.add, op1=ALU.max)
                v.tensor_scalar_min(f1t[m], f1f, 6.0)
            ft = pa.tile([128, KE, C], F16, tag="a4")
            for m in range(KE):
                pf2 = ps.tile([128, C], F32, tag="mm")
                for k in range(2):
                    mm(pf2, fw2t[:, k, m * 128:(m + 1) * 128],
                       f1t[k], start=(k == 0), stop=(k == 1))
                sc.activation(ft[:, m, :], pf2, AF.Identity,
                              bias=fb2_col[:, m:m + 1])
            frt = pa.tile([128, KE, C], F16, tag="a4")
            ln_pair([(ft, flng_col, flnb_col, frt)], relu=True, apply_dve=True)
            p1 = ps.tile([128, C], F32, tag="mm")
            for k in range(KE):
                mm(p1, rw1t[:, k, :], frt[:, k, :], start=(k == 0),
                   stop=(k == KE - 1))
            h1f = pc.tile([128, C], F16, tag="c1")
            v.tensor_scalar(out=h1f, in0=p1, scalar1=rb1_col[:, 0:1],
                            scalar2=0.0, op0=ALU.add, op1=ALU.max)
            h1t = pc.tile([128, C], F16, tag="c1")
            v.tensor_scalar_min(h1t, h1f, 6.0)
            p2 = ps.tile([E8, C], F32, tag="mm")
            mm(p2, rw2t, h1t, start=True, stop=True)
            h2f = pc.tile([E8, C], F16, tag="c1")
            sc.activation(h2f, p2, AF.Relu, bias=rb2_col[:, 0:1])
            h2t = pc.tile([E8, C], F16, tag="c1")
            v.tensor_scalar_min(h2t, h2f, 6.0)
            ot = pc.tile([128, NTT, c.OUT], F32, tag="c2", bufs=2)
            for tt in range(NTT):
                p3 = ps.tile([128, 16], F32, tag="mm")
                mm(p3, h2t[:, tt * 128:(tt + 1) * 128], rw3t,
                   start=True, stop=False)
                mm(p3, ONES_ROW[:, 0:128], rb3_row, start=False, stop=True)
                sc.activation(ot[:, tt, :], p3[:, 0:c.OUT], AF.Copy)
            nc.sync.dma_start(
                out=out_ap[ci * C:(ci + 1) * C, :].rearrange(
                    "(tt p) o -> p tt o", p=128),
                in_=ot)

    return din, out_dram


# ======================================================================
# kernel() entry point: full inputs in, full outputs out (8-core SPMD).
# ======================================================================
import concourse.bacc as _bacc
from concourse.bass_utils import run_bass_kernel_spmd as _run_spmd

_N_CORES = 8
_CACHE = {}


def _get_nc():
    if "nc" not in _CACHE:
        nc = _bacc.Bacc("TRN2", target_bir_lowering=False, debug=False)
        build(nc, Cfg())
        nc.finalize()
        _CACHE["nc"] = nc
    return _CACHE["nc"]


def kernel(**inputs):
    nc = _get_nc()
    cfg = Cfg()
    arr = {k: np.asarray(v) for k, v in inputs.items()}
    consts = host_constants(cfg, arr)
    shared = {k: a for k, a in consts.items()
              if k not in ("body_feats", "limb_feats")}
    in_maps = []
    for i in range(_N_CORES):
        m = dict(shared)
        m["body_feats"] = np.ascontiguousarray(consts["body_feats"][i])
        m["limb_feats"] = np.ascontiguousarray(consts["limb_feats"][i])
        in_maps.append(m)
    res = run_kernel_spmd_cached(nc, in_maps)
    out = np.stack([res[i]["out"] for i in range(_N_CORES)], axis=0)
    return out.astype(np.float32)


def run_kernel_spmd_cached(nc, in_maps, **kw):
    r = _run_spmd(nc, in_maps, list(range(_N_CORES)), **kw)
    _CACHE["last_result"] = r
    return r.results
